# revision 8
# baseline (speedup 1.0000x reference)
"""Trainium2 Bass kernel for the DWN block:
LayerNorm -> LRU (complex diagonal scan) -> GELU -> Linear(d,2d) -> GLU -> +x.

Strategy:
- Data-parallel: 1 batch element per NeuronCore (8 cores), SPMD NEFF.
- Transposed on-device layout [feature, time]: every matmul contracts the
  partition axis directly, and the LRU scan runs along the free axis.
- Complex scan decoupling: with lam = r*e^{i*theta} per state,
  u_t := e^{-i*theta*t} x_t obeys u_t = r*u_{t-1} + e^{-i*theta*t} b_t,
  i.e. two independent REAL first-order scans (re/im) per state ->
  hardware tensor_tensor_scan along the free axis. Twiddle factors
  cos/sin(theta*t) are precomputed on host in float64.
- LayerNorm stats via all-ones matmuls on the tensor engine (result
  replicated across all 128 partitions); ln_w/ln_b folded into the
  downstream weights/biases on host.
- Matmul operands in fp16 (fp32 PSUM accumulation, 1 cyc/row on PE);
  scan decay r, LN stats, GLU and residual stay fp32.
"""

import numpy as np

import concourse.bacc as bacc
import concourse.tile as tile
from concourse import mybir
from concourse import bass_utils

# ---- problem constants (hardcoded per contract) ----
B, L, D, S = 8, 2048, 512, 256
DFF = 2 * D
LN_EPS = 1e-5
N_CORES = 8

# ---- tiling ----
P = 128
TC = 512                 # time chunk
NCHUNK = L // TC         # 4
KD = D // P              # 4  k-tiles over d
KS = S // P              # 2  k-tiles over s
MD = D // P              # 4  m-tiles over d outputs

F32 = mybir.dt.float32
F16 = mybir.dt.float16
AOP = mybir.AluOpType
AF = mybir.ActivationFunctionType
NP16 = np.float16


def _pack_kpm(w, k_tiles, m):
    """[K, M] -> [128, k_tiles, M] host pack for lhsT storage (K = kt*128+p)."""
    K = k_tiles * P
    assert w.shape == (K, m)
    return np.ascontiguousarray(w.reshape(k_tiles, P, m).transpose(1, 0, 2))


def _build(nc, with_bc=False):
    f32 = F32
    f16 = F16

    xT = nc.dram_tensor("xT", [P, KD, L], f32, kind="ExternalInput")
    bt_re = nc.dram_tensor("bt_re", [P, KD, S], f16, kind="ExternalInput")
    bt_im = nc.dram_tensor("bt_im", [P, KD, S], f16, kind="ExternalInput")
    ct_re = nc.dram_tensor("ct_re", [P, KS, D], f16, kind="ExternalInput")
    ct_imn = nc.dram_tensor("ct_imn", [P, KS, D], f16, kind="ExternalInput")
    dt_w = nc.dram_tensor("dt_w", [P, KD, D], f16, kind="ExternalInput")
    wt = nc.dram_tensor("wt", [P, KD, DFF], f16, kind="ExternalInput")
    cosT = nc.dram_tensor("cosT", [P, KS, L], f16, kind="ExternalInput")
    sinT = nc.dram_tensor("sinT", [P, KS, L], f16, kind="ExternalInput")
    r_b = nc.dram_tensor("r_b", [P, KS, TC], f32, kind="ExternalInput")
    bc_re = nc.dram_tensor("bc_re", [P, KS], f32, kind="ExternalInput")
    bc_im = nc.dram_tensor("bc_im", [P, KS], f32, kind="ExternalInput")
    gbias = nc.dram_tensor("gbias", [P, MD], f32, kind="ExternalInput")
    b_a = nc.dram_tensor("b_a", [P, MD], f32, kind="ExternalInput")
    b_g = nc.dram_tensor("b_g", [P, MD], f32, kind="ExternalInput")
    outT = nc.dram_tensor("outT", [P, KD, L], f32, kind="ExternalOutput")

    with tile.TileContext(nc) as tc:
        with (
            tc.tile_pool(name="wpool", bufs=1) as wpool,
            tc.tile_pool(name="io", bufs=2) as io,
            tc.tile_pool(name="work", bufs=1) as work,
            tc.tile_pool(name="carry", bufs=2) as carry_pool,
            tc.tile_pool(name="psum", bufs=1, space="PSUM") as psum,
        ):
            # ---- resident weights/constants ----
            w_bt_re = wpool.tile([P, KD, S], f16)
            nc.sync.dma_start(w_bt_re[:], bt_re[:])
            w_bt_im = wpool.tile([P, KD, S], f16)
            nc.sync.dma_start(w_bt_im[:], bt_im[:])
            w_ct_re = wpool.tile([P, KS, D], f16)
            nc.sync.dma_start(w_ct_re[:], ct_re[:])
            w_ct_imn = wpool.tile([P, KS, D], f16)
            nc.sync.dma_start(w_ct_imn[:], ct_imn[:])
            w_dt = wpool.tile([P, KD, D], f16)
            nc.sync.dma_start(w_dt[:], dt_w[:])
            w_wt = wpool.tile([P, KD, DFF], f16)
            nc.sync.dma_start(w_wt[:], wt[:])
            w_r = wpool.tile([P, KS, TC], f32)
            nc.sync.dma_start(w_r[:], r_b[:])
            w_bc_re = wpool.tile([P, KS], f32)
            nc.sync.dma_start(w_bc_re[:], bc_re[:])
            w_bc_im = wpool.tile([P, KS], f32)
            nc.sync.dma_start(w_bc_im[:], bc_im[:])
            w_gbias = wpool.tile([P, MD], f32)
            nc.sync.dma_start(w_gbias[:], gbias[:])
            w_ba = wpool.tile([P, MD], f32)
            nc.sync.dma_start(w_ba[:], b_a[:])
            w_bg = wpool.tile([P, MD], f32)
            nc.sync.dma_start(w_bg[:], b_g[:])
            ones = wpool.tile([P, P], f16)
            nc.vector.memset(ones, 1.0)
            w_eps = wpool.tile([P, 1], f32)
            nc.vector.memset(w_eps, LN_EPS)

            u_prev = None
            for ck in range(NCHUNK):
                t0 = ck * TC

                # ---- load x chunk + twiddle chunk ----
                x_sb = io.tile([P, KD, TC], f32, tag="x")
                nc.sync.dma_start(x_sb[:], xT[:, :, t0 : t0 + TC])
                cos_sb = io.tile([P, KS, TC], f16, tag="cos")
                nc.sync.dma_start(cos_sb[:], cosT[:, :, t0 : t0 + TC])
                sin_sb = io.tile([P, KS, TC], f16, tag="sin")
                nc.sync.dma_start(sin_sb[:], sinT[:, :, t0 : t0 + TC])

                # ---- LN stats: mu_rep / msq_rep via all-ones matmuls ----
                x16_sb = work.tile([P, KD, TC], f16, tag="x16")
                x2_sb = work.tile([P, KD, TC], f16, tag="x2")
                for kt in range(KD):
                    nc.vector.tensor_copy(x16_sb[:, kt, :], x_sb[:, kt, :])
                    nc.scalar.activation(x2_sb[:, kt, :], x_sb[:, kt, :], AF.Square)
                mu_ps = psum.tile([P, TC], f32, tag="A", bufs=4, name="mu_ps")
                msq_ps = psum.tile([P, TC], f32, tag="A", bufs=4, name="msq_ps")
                for kt in range(KD):
                    nc.tensor.matmul(
                        mu_ps[:], lhsT=ones[:], rhs=x16_sb[:, kt, :],
                        start=(kt == 0), stop=(kt == KD - 1),
                    )
                for kt in range(KD):
                    nc.tensor.matmul(
                        msq_ps[:], lhsT=ones[:], rhs=x2_sb[:, kt, :],
                        start=(kt == 0), stop=(kt == KD - 1),
                    )

                # ---- LN apply ----
                mu2 = work.tile([P, TC], f32, tag="mu2")
                nc.scalar.activation(mu2[:], mu_ps[:], AF.Square, scale=1.0 / D)
                var = work.tile([P, TC], f32, tag="var")
                nc.vector.scalar_tensor_tensor(
                    var[:], msq_ps[:], 1.0 / D, mu2[:],
                    op0=AOP.mult, op1=AOP.subtract,
                )
                rstd = work.tile([P, TC], f32, tag="rstd")
                nc.scalar.activation(rstd[:], var[:], AF.Sqrt, bias=w_eps[:])
                nc.vector.reciprocal(rstd[:], rstd[:])
                mb = work.tile([P, TC], f32, tag="mb")
                nc.vector.scalar_tensor_tensor(
                    mb[:], mu_ps[:], 1.0 / D, rstd[:],
                    op0=AOP.mult, op1=AOP.mult,
                )
                xhat = work.tile([P, KD, TC], f16, tag="xhat")
                xh32 = work.tile([P, KD, TC], f32, tag="xh32")
                for kt in range(KD):
                    nc.gpsimd.tensor_mul(xh32[:, kt, :], x_sb[:, kt, :], rstd[:])
                    nc.gpsimd.tensor_sub(xhat[:, kt, :], xh32[:, kt, :], mb[:])

                # ---- Bu matmuls -> psum (4 banks: [re/im] x [s-tile]) ----
                ps_bu = [
                    [
                        psum.tile([P, TC], f32, tag="B", bufs=4, name=f"bu{c}{st}")
                        for st in range(KS)
                    ]
                    for c in range(2)
                ]
                for st in range(KS):
                    for comp, w_bt in ((0, w_bt_re), (1, w_bt_im)):
                        for kt in range(KD):
                            nc.tensor.matmul(
                                ps_bu[comp][st][:],
                                lhsT=w_bt[:, kt, st * P : (st + 1) * P],
                                rhs=xhat[:, kt, :],
                                start=(kt == 0),
                                stop=(kt == KD - 1),
                            )

                # ---- evac Bu (+ state bias bc = B_norm @ ln_b) to fp16 ----
                bu_re = work.tile([P, KS, TC], f16, tag="bu_re")
                bu_im = work.tile([P, KS, TC], f16, tag="bu_im")
                for st in range(KS):
                    if with_bc:
                        nc.vector.tensor_scalar_add(
                            bu_re[:, st, :], ps_bu[0][st][:],
                            w_bc_re[:, st : st + 1],
                        )
                        nc.vector.tensor_scalar_add(
                            bu_im[:, st, :], ps_bu[1][st][:],
                            w_bc_im[:, st : st + 1],
                        )
                    else:
                        nc.scalar.activation(
                            bu_re[:, st, :], ps_bu[0][st][:], AF.Copy)
                        nc.scalar.activation(
                            bu_im[:, st, :], ps_bu[1][st][:], AF.Copy)

                # ---- twiddle: c = e^{-i theta t} * Bu ----
                c_re = work.tile([P, KS, TC], f16, tag="c_re")
                c_im = work.tile([P, KS, TC], f16, tag="c_im")
                tw1 = work.tile([P, KS, TC], f16, tag="tw1")
                tw2 = work.tile([P, KS, TC], f16, tag="tw2")
                for st in range(KS):
                    nc.vector.tensor_mul(tw1[:, st, :], cos_sb[:, st, :], bu_re[:, st, :])
                    nc.vector.tensor_mul(tw2[:, st, :], sin_sb[:, st, :], bu_im[:, st, :])
                    nc.vector.tensor_add(c_re[:, st, :], tw1[:, st, :], tw2[:, st, :])
                    nc.vector.tensor_mul(tw1[:, st, :], cos_sb[:, st, :], bu_im[:, st, :])
                    nc.vector.tensor_mul(tw2[:, st, :], sin_sb[:, st, :], bu_re[:, st, :])
                    nc.vector.tensor_sub(c_im[:, st, :], tw1[:, st, :], tw2[:, st, :])

                # ---- scans: u_t = r*u_{t-1} + c_t (re/im per s-tile) ----
                u = carry_pool.tile([P, 2, KS, TC], f16, tag="u")
                for comp, c_t in ((0, c_re), (1, c_im)):
                    for st in range(KS):
                        init = (
                            0.0 if u_prev is None
                            else u_prev[:, comp, st, TC - 1 : TC]
                        )
                        nc.vector.tensor_tensor_scan(
                            u[:, comp, st, :],
                            w_r[:, st, :],
                            c_t[:, st, :],
                            init,
                            op0=AOP.mult,
                            op1=AOP.add,
                        )
                u_prev = u

                # ---- untwiddle: xs = e^{+i theta t} u ----
                xs_re = work.tile([P, KS, TC], f16, tag="xs_re")
                xs_im = work.tile([P, KS, TC], f16, tag="xs_im")
                for st in range(KS):
                    nc.vector.tensor_mul(tw1[:, st, :], cos_sb[:, st, :], u[:, 0, st, :])
                    nc.vector.tensor_mul(tw2[:, st, :], sin_sb[:, st, :], u[:, 1, st, :])
                    nc.vector.tensor_sub(xs_re[:, st, :], tw1[:, st, :], tw2[:, st, :])
                    nc.vector.tensor_mul(tw1[:, st, :], sin_sb[:, st, :], u[:, 0, st, :])
                    nc.vector.tensor_mul(tw2[:, st, :], cos_sb[:, st, :], u[:, 1, st, :])
                    nc.vector.tensor_add(xs_im[:, st, :], tw1[:, st, :], tw2[:, st, :])

                # ---- y = C_re@xs_re + (-C_im)@xs_im + (D.w)@xhat -> gelu ----
                h_sb = work.tile([P, MD, TC], f16, tag="h")
                for mt in range(MD):
                    ps_y = psum.tile([P, TC], f32, tag="B", bufs=4, name=f"y{mt}")
                    for st in range(KS):
                        nc.tensor.matmul(
                            ps_y[:],
                            lhsT=w_ct_re[:, st, mt * P : (mt + 1) * P],
                            rhs=xs_re[:, st, :],
                            start=(st == 0), stop=False,
                        )
                    for st in range(KS):
                        nc.tensor.matmul(
                            ps_y[:],
                            lhsT=w_ct_imn[:, st, mt * P : (mt + 1) * P],
                            rhs=xs_im[:, st, :],
                            start=False, stop=False,
                        )
                    for kt in range(KD):
                        nc.tensor.matmul(
                            ps_y[:],
                            lhsT=w_dt[:, kt, mt * P : (mt + 1) * P],
                            rhs=xhat[:, kt, :],
                            start=False, stop=(kt == KD - 1),
                        )
                    nc.scalar.activation(
                        h_sb[:, mt, :], ps_y[:], AF.Gelu,
                        bias=w_gbias[:, mt : mt + 1],
                    )

                # ---- proj = W.h ; GLU ; residual ----
                out_sb = io.tile([P, KD, TC], f32, tag="out")
                sig = work.tile([P, MD, TC], f32, tag="sig")
                for mt in range(MD):
                    ps_pa = psum.tile([P, TC], f32, tag="A", bufs=4, name=f"pa{mt}")
                    ps_pg = psum.tile([P, TC], f32, tag="A", bufs=4, name=f"pg{mt}")
                    for kt in range(KD):
                        nc.tensor.matmul(
                            ps_pa[:],
                            lhsT=w_wt[:, kt, mt * P : (mt + 1) * P],
                            rhs=h_sb[:, kt, :],
                            start=(kt == 0), stop=(kt == KD - 1),
                        )
                    for kt in range(KD):
                        nc.tensor.matmul(
                            ps_pg[:],
                            lhsT=w_wt[:, kt, D + mt * P : D + (mt + 1) * P],
                            rhs=h_sb[:, kt, :],
                            start=(kt == 0), stop=(kt == KD - 1),
                        )
                    nc.scalar.activation(
                        sig[:, mt, :], ps_pg[:], AF.Sigmoid,
                        bias=w_bg[:, mt : mt + 1],
                    )
                    nc.vector.scalar_tensor_tensor(
                        out_sb[:, mt, :], ps_pa[:], w_ba[:, mt : mt + 1],
                        sig[:, mt, :], op0=AOP.add, op1=AOP.mult,
                    )
                    nc.gpsimd.tensor_add(
                        out_sb[:, mt, :], out_sb[:, mt, :], x_sb[:, mt, :]
                    )

                nc.sync.dma_start(outT[:, :, t0 : t0 + TC], out_sb[:])

    nc.compile()
    return nc


_NC_CACHE = {}


def _get_module(with_bc=False):
    if with_bc not in _NC_CACHE:
        nc = bacc.Bacc("TRN2", target_bir_lowering=False, debug=False)
        _NC_CACHE[with_bc] = _build(nc, with_bc=with_bc)
    return _NC_CACHE[with_bc]


def _host_prepack(inputs):
    ln_w = np.asarray(inputs["ln_w"], np.float64)
    ln_b = np.asarray(inputs["ln_b"], np.float64)
    nu_log = np.asarray(inputs["nu_log"], np.float64)
    theta_log = np.asarray(inputs["theta_log"], np.float64)
    gamma_log = np.asarray(inputs["gamma_log"], np.float64)
    B_re = np.asarray(inputs["B_re"], np.float64)
    B_im = np.asarray(inputs["B_im"], np.float64)
    C_re = np.asarray(inputs["C_re"], np.float64)
    C_im = np.asarray(inputs["C_im"], np.float64)
    D_m = np.asarray(inputs["D"], np.float64)
    W_out = np.asarray(inputs["W_out"], np.float64)
    b_out = np.asarray(inputs["b_out"], np.float64)

    r = np.exp(-np.exp(nu_log))
    theta = np.exp(theta_log)
    g = np.exp(gamma_log)
    ang = theta[:, None] * np.arange(L, dtype=np.float64)[None, :]
    cos_t = np.cos(ang)
    sin_t = np.sin(ang)

    Bn_re = B_re * g[:, None]
    Bn_im = B_im * g[:, None]
    BnT_re = (Bn_re * ln_w[None, :]).T
    BnT_im = (Bn_im * ln_w[None, :]).T
    bc_re_v = Bn_re @ ln_b
    bc_im_v = Bn_im @ ln_b
    CT_re = C_re.T
    CT_imn = (-C_im).T
    DT = (D_m * ln_w[None, :]).T
    gbias_v = D_m @ ln_b
    WT = W_out.T
    b_a_v = b_out[:D]
    b_g_v = b_out[D:]

    def cols(v, ntiles):
        return np.ascontiguousarray(np.asarray(v, np.float32).reshape(ntiles, P).T)

    return {
        "bt_re": _pack_kpm(BnT_re, KD, S).astype(NP16),
        "bt_im": _pack_kpm(BnT_im, KD, S).astype(NP16),
        "ct_re": _pack_kpm(CT_re, KS, D).astype(NP16),
        "ct_imn": _pack_kpm(CT_imn, KS, D).astype(NP16),
        "dt_w": _pack_kpm(DT, KD, D).astype(NP16),
        "wt": _pack_kpm(WT, KD, DFF).astype(NP16),
        "cosT": np.ascontiguousarray(
            cos_t.reshape(KS, P, L).transpose(1, 0, 2)
        ).astype(NP16),
        "sinT": np.ascontiguousarray(
            sin_t.reshape(KS, P, L).transpose(1, 0, 2)
        ).astype(NP16),
        "r_b": np.ascontiguousarray(
            np.broadcast_to(r.reshape(KS, P, 1), (KS, P, TC)).transpose(1, 0, 2)
        ).astype(np.float32),
        "bc_re": cols(bc_re_v, KS),
        "bc_im": cols(bc_im_v, KS),
        "gbias": cols(gbias_v, MD),
        "b_a": cols(b_a_v, MD),
        "b_g": cols(b_g_v, MD),
    }


def kernel(**inputs):
    x = np.asarray(inputs["x"], np.float32)
    weights = _host_prepack(inputs)

    in_maps = []
    for b in range(B):
        xb = np.ascontiguousarray(x[b].T.reshape(KD, P, L).transpose(1, 0, 2))
        m = dict(weights)
        m["xT"] = xb
        in_maps.append(m)

    with_bc = bool(np.any(np.asarray(inputs["ln_b"]) != 0))
    nc = _get_module(with_bc)
    res = bass_utils.run_bass_kernel_spmd(nc, in_maps, core_ids=list(range(N_CORES)))
    out = np.empty((B, L, D), np.float32)
    for b in range(B):
        ob = res.results[b]["outT"]
        out[b] = ob.transpose(1, 0, 2).reshape(D, L).T
    return out


# revision 14
# speedup vs baseline: 1.0967x; 1.0967x over previous
"""Trainium2 Bass kernel for the DWN block:
LayerNorm -> LRU (complex diagonal scan) -> GELU -> Linear(d,2d) -> GLU -> +x.

Strategy:
- Data-parallel: 1 batch element per NeuronCore (8 cores), SPMD NEFF.
- Transposed on-device layout [feature, time]: every matmul contracts the
  partition axis directly, and the LRU scan runs along the free axis.
- Complex scan decoupling: with lam = r*e^{i*theta} per state,
  u_t := e^{-i*theta*t} x_t obeys u_t = r*u_{t-1} + e^{-i*theta*t} b_t,
  i.e. two independent REAL first-order scans (re/im) per state ->
  hardware tensor_tensor_scan along the free axis. Twiddle factors
  cos/sin(theta*t) are precomputed on host in float64.
- LayerNorm stats for ALL time chunks are computed in a prologue via
  all-ones matmuls (result replicated across partitions); one batched
  Sqrt + fast-reciprocal gives rstd. ln_w/ln_b are folded into the
  downstream weights/biases on host.
- Matmul operands fp16 (fp32 PSUM accumulation, 1 cyc/row); scan decay
  r, GLU and residual fp32.
- ScalarE stays on the gelu_and_others table set (gelu/tanh/square/copy):
  sigmoid(g) is computed as 0.5 + 0.5*tanh(g/2) folded into the GLU math,
  so only ~2 ACT table loads happen for the whole kernel.
"""

import numpy as np

import concourse.bacc as bacc
import concourse.tile as tile
from concourse import mybir
from concourse import bass_utils

# ---- problem constants (hardcoded per contract) ----
B, L, D, S = 8, 2048, 512, 256
DFF = 2 * D
LN_EPS = 1e-5
N_CORES = 8

# ---- tiling ----
P = 128
TC = 512                 # time chunk
NCHUNK = L // TC         # 4
KD = D // P              # 4  k-tiles over d
KS = S // P              # 2  k-tiles over s
MD = D // P              # 4  m-tiles over d outputs

F32 = mybir.dt.float32
F16 = mybir.dt.float16
AOP = mybir.AluOpType
AF = mybir.ActivationFunctionType
NP16 = np.float16


def _pack_kpm(w, k_tiles, m):
    """[K, M] -> [128, k_tiles, M] host pack for lhsT storage (K = kt*128+p)."""
    K = k_tiles * P
    assert w.shape == (K, m)
    return np.ascontiguousarray(w.reshape(k_tiles, P, m).transpose(1, 0, 2))


def _build(nc, with_bc=False):
    f32 = F32
    f16 = F16

    xT = nc.dram_tensor("xT", [P, KD, L], f32, kind="ExternalInput")
    xT16 = nc.dram_tensor("xT16", [P, KD, L], f16, kind="ExternalInput")
    bt_re = nc.dram_tensor("bt_re", [P, KD, S], f16, kind="ExternalInput")
    bt_im = nc.dram_tensor("bt_im", [P, KD, S], f16, kind="ExternalInput")
    ct_re = nc.dram_tensor("ct_re", [P, KS, D], f16, kind="ExternalInput")
    ct_imn = nc.dram_tensor("ct_imn", [P, KS, D], f16, kind="ExternalInput")
    dt_w = nc.dram_tensor("dt_w", [P, KD, D], f16, kind="ExternalInput")
    wt = nc.dram_tensor("wt", [P, KD, DFF], f16, kind="ExternalInput")
    cosT = nc.dram_tensor("cosT", [P, KS, L], f16, kind="ExternalInput")
    sinT = nc.dram_tensor("sinT", [P, KS, L], f16, kind="ExternalInput")
    r_b = nc.dram_tensor("r_b", [P, KS, TC], f32, kind="ExternalInput")
    bc_re = nc.dram_tensor("bc_re", [P, KS], f32, kind="ExternalInput")
    bc_im = nc.dram_tensor("bc_im", [P, KS], f32, kind="ExternalInput")
    gbias = nc.dram_tensor("gbias", [P, MD], f32, kind="ExternalInput")
    b_a = nc.dram_tensor("b_a", [P, MD], f32, kind="ExternalInput")
    b_gh = nc.dram_tensor("b_gh", [P, MD], f32, kind="ExternalInput")
    outT = nc.dram_tensor("outT", [P, KD, L], f32, kind="ExternalOutput")

    with tile.TileContext(nc) as tc:
        with (
            tc.tile_pool(name="wpool", bufs=1) as wpool,
            tc.tile_pool(name="io", bufs=2) as io,
            tc.tile_pool(name="work", bufs=1) as work,
            tc.tile_pool(name="carry", bufs=2) as carry_pool,
            tc.tile_pool(name="psum", bufs=1, space="PSUM") as psum,
        ):
            # ---- resident weights/constants ----
            w_bt_re = wpool.tile([P, KD, S], f16)
            nc.sync.dma_start(w_bt_re[:], bt_re[:])
            w_bt_im = wpool.tile([P, KD, S], f16)
            nc.sync.dma_start(w_bt_im[:], bt_im[:])
            w_ct_re = wpool.tile([P, KS, D], f16)
            nc.sync.dma_start(w_ct_re[:], ct_re[:])
            w_ct_imn = wpool.tile([P, KS, D], f16)
            nc.sync.dma_start(w_ct_imn[:], ct_imn[:])
            w_dt = wpool.tile([P, KD, D], f16)
            nc.sync.dma_start(w_dt[:], dt_w[:])
            w_wt = wpool.tile([P, KD, DFF], f16)
            nc.sync.dma_start(w_wt[:], wt[:])
            w_r = wpool.tile([P, KS, TC], f32)
            nc.sync.dma_start(w_r[:], r_b[:])
            w_bc_re = wpool.tile([P, KS], f32)
            nc.sync.dma_start(w_bc_re[:], bc_re[:])
            w_bc_im = wpool.tile([P, KS], f32)
            nc.sync.dma_start(w_bc_im[:], bc_im[:])
            w_gbias = wpool.tile([P, MD], f32)
            nc.sync.dma_start(w_gbias[:], gbias[:])
            w_ba = wpool.tile([P, MD], f32)
            nc.sync.dma_start(w_ba[:], b_a[:])
            w_bgh = wpool.tile([P, MD], f32)
            nc.sync.dma_start(w_bgh[:], b_gh[:])
            ones = wpool.tile([P, P], f16)
            nc.vector.memset(ones, 1.0)
            w_eps = wpool.tile([P, 1], f32)
            nc.vector.memset(w_eps, LN_EPS)

            # ---- phase 0: LN stats for all chunks ----
            # x16 for the whole sequence stays resident (feeds xc later).
            x16_sb = wpool.tile([P, KD, L], f16)
            nc.sync.dma_start(x16_sb[:], xT16[:])
            mu16_all = wpool.tile([P, NCHUNK, TC], f16)
            var_all = wpool.tile([P, NCHUNK, TC], f32)
            for ck in range(NCHUNK):
                t0 = ck * TC
                x2_sb = work.tile([P, KD, TC], f16, tag="x2", bufs=2)
                for kt in range(KD):
                    nc.scalar.activation(
                        x2_sb[:, kt, :], x16_sb[:, kt, t0 : t0 + TC], AF.Square
                    )
                mu_ps = psum.tile([P, TC], f32, tag="A", bufs=4, name=f"mu{ck}")
                msq_ps = psum.tile([P, TC], f32, tag="A", bufs=4, name=f"msq{ck}")
                for kt in range(KD):
                    nc.tensor.matmul(
                        mu_ps[:], lhsT=ones[:], rhs=x16_sb[:, kt, t0 : t0 + TC],
                        start=(kt == 0), stop=(kt == KD - 1),
                    )
                for kt in range(KD):
                    nc.tensor.matmul(
                        msq_ps[:], lhsT=ones[:], rhs=x2_sb[:, kt, :],
                        start=(kt == 0), stop=(kt == KD - 1),
                    )
                # mu' (fp16, for xc) and var = msq/D - mu'^2
                nc.scalar.activation(
                    mu16_all[:, ck, :], mu_ps[:], AF.Copy, scale=1.0 / D
                )
                mu2 = work.tile([P, TC], f32, tag="mu2")
                nc.scalar.activation(mu2[:], mu_ps[:], AF.Square, scale=1.0 / D)
                nc.vector.scalar_tensor_tensor(
                    var_all[:, ck, :], msq_ps[:], 1.0 / D, mu2[:],
                    op0=AOP.mult, op1=AOP.subtract,
                )
            # sigma = sqrt(var + eps) over all chunks at once, then
            # rstd = 1/sigma via the fast custom-DVE reciprocal (~18 bits).
            sigma_all = work.tile([P, NCHUNK * TC], f32, tag="sigma", bufs=1)
            nc.scalar.activation(
                sigma_all[:], var_all.rearrange("p c t -> p (c t)"),
                AF.Sqrt, bias=w_eps[:],
            )
            rstd32 = work.tile([P, NCHUNK * TC], f32, tag="rstd32", bufs=1)
            nc.vector.reciprocal_approx_fast(rstd32[:], sigma_all[:])
            rstd16_all = wpool.tile([P, NCHUNK, TC], f16)
            nc.scalar.activation(
                rstd16_all.rearrange("p c t -> p (c t)"), rstd32[:], AF.Copy
            )

            # ---- main loop over time chunks ----
            u_prev = None
            for ck in range(NCHUNK):
                t0 = ck * TC

                x_sb = io.tile([P, KD, TC], f32, tag="x")
                nc.sync.dma_start(x_sb[:], xT[:, :, t0 : t0 + TC])
                cos_sb = io.tile([P, KS, TC], f16, tag="cos")
                nc.sync.dma_start(cos_sb[:], cosT[:, :, t0 : t0 + TC])
                sin_sb = io.tile([P, KS, TC], f16, tag="sin")
                nc.sync.dma_start(sin_sb[:], sinT[:, :, t0 : t0 + TC])

                # ---- LN apply: xhat = (x - mu')*rstd, all fp16 ----
                xc = work.tile([P, KD, TC], f16, tag="xc")
                xhat = work.tile([P, KD, TC], f16, tag="xhat")
                for kt in range(KD):
                    nc.vector.tensor_sub(
                        xc[:, kt, :], x16_sb[:, kt, t0 : t0 + TC],
                        mu16_all[:, ck, :],
                    )
                    nc.vector.tensor_mul(
                        xhat[:, kt, :], xc[:, kt, :], rstd16_all[:, ck, :]
                    )

                # ---- Bu matmuls -> psum (4 banks: [re/im] x [s-tile]) ----
                ps_bu = [
                    [
                        psum.tile([P, TC], f32, tag="B", bufs=4, name=f"bu{c}{st}")
                        for st in range(KS)
                    ]
                    for c in range(2)
                ]
                for st in range(KS):
                    for comp, w_bt in ((0, w_bt_re), (1, w_bt_im)):
                        for kt in range(KD):
                            nc.tensor.matmul(
                                ps_bu[comp][st][:],
                                lhsT=w_bt[:, kt, st * P : (st + 1) * P],
                                rhs=xhat[:, kt, :],
                                start=(kt == 0),
                                stop=(kt == KD - 1),
                            )

                # ---- evac Bu (+ state bias bc = B_norm @ ln_b) to fp16 ----
                bu_re = work.tile([P, KS, TC], f16, tag="bu_re")
                bu_im = work.tile([P, KS, TC], f16, tag="bu_im")
                for st in range(KS):
                    if with_bc:
                        nc.vector.tensor_scalar_add(
                            bu_re[:, st, :], ps_bu[0][st][:],
                            w_bc_re[:, st : st + 1],
                        )
                        nc.vector.tensor_scalar_add(
                            bu_im[:, st, :], ps_bu[1][st][:],
                            w_bc_im[:, st : st + 1],
                        )
                    else:
                        nc.scalar.activation(
                            bu_re[:, st, :], ps_bu[0][st][:], AF.Copy)
                        nc.scalar.activation(
                            bu_im[:, st, :], ps_bu[1][st][:], AF.Copy)

                # ---- twiddle: c = e^{-i theta t} * Bu ----
                c_re = work.tile([P, KS, TC], f16, tag="c_re")
                c_im = work.tile([P, KS, TC], f16, tag="c_im")
                tw1 = work.tile([P, KS, TC], f16, tag="tw1")
                tw2 = work.tile([P, KS, TC], f16, tag="tw2")
                for st in range(KS):
                    nc.vector.tensor_mul(tw1[:, st, :], cos_sb[:, st, :], bu_re[:, st, :])
                    nc.vector.tensor_mul(tw2[:, st, :], sin_sb[:, st, :], bu_im[:, st, :])
                    nc.vector.tensor_add(c_re[:, st, :], tw1[:, st, :], tw2[:, st, :])
                    nc.vector.tensor_mul(tw1[:, st, :], cos_sb[:, st, :], bu_im[:, st, :])
                    nc.vector.tensor_mul(tw2[:, st, :], sin_sb[:, st, :], bu_re[:, st, :])
                    nc.vector.tensor_sub(c_im[:, st, :], tw1[:, st, :], tw2[:, st, :])

                # ---- scans: u_t = r*u_{t-1} + c_t (re/im per s-tile) ----
                u = carry_pool.tile([P, 2, KS, TC], f16, tag="u")
                for comp, c_t in ((0, c_re), (1, c_im)):
                    for st in range(KS):
                        init = (
                            0.0 if u_prev is None
                            else u_prev[:, comp, st, TC - 1 : TC]
                        )
                        nc.vector.tensor_tensor_scan(
                            u[:, comp, st, :],
                            w_r[:, st, :],
                            c_t[:, st, :],
                            init,
                            op0=AOP.mult,
                            op1=AOP.add,
                        )
                u_prev = u

                # ---- untwiddle: xs = e^{+i theta t} u ----
                # s-tile 0 on VectorE, s-tile 1 on GpSimd (load balance)
                xs_re = work.tile([P, KS, TC], f16, tag="xs_re")
                xs_im = work.tile([P, KS, TC], f16, tag="xs_im")
                tg1 = work.tile([P, TC], f16, tag="tg1")
                tg2 = work.tile([P, TC], f16, tag="tg2")
                for st in range(KS):
                    eng = nc.vector if st == 0 else nc.gpsimd
                    t1 = tw1[:, st, :] if st == 0 else tg1[:]
                    t2 = tw2[:, st, :] if st == 0 else tg2[:]
                    eng.tensor_mul(t1, cos_sb[:, st, :], u[:, 0, st, :])
                    eng.tensor_mul(t2, sin_sb[:, st, :], u[:, 1, st, :])
                    eng.tensor_sub(xs_re[:, st, :], t1, t2)
                    eng.tensor_mul(t1, sin_sb[:, st, :], u[:, 0, st, :])
                    eng.tensor_mul(t2, cos_sb[:, st, :], u[:, 1, st, :])
                    eng.tensor_add(xs_im[:, st, :], t1, t2)

                # ---- y = C_re@xs_re + (-C_im)@xs_im + (D.w)@xhat -> gelu ----
                h_sb = work.tile([P, MD, TC], f16, tag="h")
                for mt in range(MD):
                    ps_y = psum.tile([P, TC], f32, tag="B", bufs=4, name=f"y{mt}")
                    for st in range(KS):
                        nc.tensor.matmul(
                            ps_y[:],
                            lhsT=w_ct_re[:, st, mt * P : (mt + 1) * P],
                            rhs=xs_re[:, st, :],
                            start=(st == 0), stop=False,
                        )
                    for st in range(KS):
                        nc.tensor.matmul(
                            ps_y[:],
                            lhsT=w_ct_imn[:, st, mt * P : (mt + 1) * P],
                            rhs=xs_im[:, st, :],
                            start=False, stop=False,
                        )
                    for kt in range(KD):
                        nc.tensor.matmul(
                            ps_y[:],
                            lhsT=w_dt[:, kt, mt * P : (mt + 1) * P],
                            rhs=xhat[:, kt, :],
                            start=False, stop=(kt == KD - 1),
                        )
                    nc.scalar.activation(
                        h_sb[:, mt, :], ps_y[:], AF.Gelu,
                        bias=w_gbias[:, mt : mt + 1],
                    )

                # ---- proj = W.h ; GLU via tanh ; residual ----
                # sigmoid(g+bg) = 0.5 + 0.5*tanh((g+bg)/2)
                # out = (a+ba)*sig + x = 0.5*[(a+ba) + (a+ba)*t] + x
                out_sb = io.tile([P, KD, TC], f32, tag="out")
                for mt in range(MD):
                    th = work.tile([P, TC], f32, tag="th", bufs=2)
                    g1 = work.tile([P, TC], f32, tag="g1", bufs=2)
                    g2 = work.tile([P, TC], f32, tag="g2", bufs=2)
                    ps_pa = psum.tile([P, TC], f32, tag="A", bufs=4, name=f"pa{mt}")
                    ps_pg = psum.tile([P, TC], f32, tag="A", bufs=4, name=f"pg{mt}")
                    for kt in range(KD):
                        nc.tensor.matmul(
                            ps_pa[:],
                            lhsT=w_wt[:, kt, mt * P : (mt + 1) * P],
                            rhs=h_sb[:, kt, :],
                            start=(kt == 0), stop=(kt == KD - 1),
                        )
                    for kt in range(KD):
                        nc.tensor.matmul(
                            ps_pg[:],
                            lhsT=w_wt[:, kt, D + mt * P : D + (mt + 1) * P],
                            rhs=h_sb[:, kt, :],
                            start=(kt == 0), stop=(kt == KD - 1),
                        )
                    nc.scalar.activation(
                        th[:], ps_pg[:], AF.Tanh,
                        bias=w_bgh[:, mt : mt + 1], scale=0.5,
                    )
                    nc.vector.scalar_tensor_tensor(
                        g1[:], ps_pa[:], w_ba[:, mt : mt + 1],
                        th[:], op0=AOP.add, op1=AOP.mult,
                    )
                    nc.vector.scalar_tensor_tensor(
                        g2[:], ps_pa[:], w_ba[:, mt : mt + 1],
                        g1[:], op0=AOP.add, op1=AOP.add,
                    )
                    nc.gpsimd.tensor_add(
                        out_sb[:, mt, :], g2[:], x_sb[:, mt, :]
                    )

                nc.sync.dma_start(outT[:, :, t0 : t0 + TC], out_sb[:])

    nc.compile()
    return nc


_NC_CACHE = {}


def _get_module(with_bc=False):
    if with_bc not in _NC_CACHE:
        nc = bacc.Bacc("TRN2", target_bir_lowering=False, debug=False)
        _NC_CACHE[with_bc] = _build(nc, with_bc=with_bc)
    return _NC_CACHE[with_bc]


def _host_prepack(inputs):
    ln_w = np.asarray(inputs["ln_w"], np.float64)
    ln_b = np.asarray(inputs["ln_b"], np.float64)
    nu_log = np.asarray(inputs["nu_log"], np.float64)
    theta_log = np.asarray(inputs["theta_log"], np.float64)
    gamma_log = np.asarray(inputs["gamma_log"], np.float64)
    B_re = np.asarray(inputs["B_re"], np.float64)
    B_im = np.asarray(inputs["B_im"], np.float64)
    C_re = np.asarray(inputs["C_re"], np.float64)
    C_im = np.asarray(inputs["C_im"], np.float64)
    D_m = np.asarray(inputs["D"], np.float64)
    W_out = np.asarray(inputs["W_out"], np.float64)
    b_out = np.asarray(inputs["b_out"], np.float64)

    r = np.exp(-np.exp(nu_log))
    theta = np.exp(theta_log)
    g = np.exp(gamma_log)
    ang = theta[:, None] * np.arange(L, dtype=np.float64)[None, :]
    cos_t = np.cos(ang)
    sin_t = np.sin(ang)

    Bn_re = B_re * g[:, None]
    Bn_im = B_im * g[:, None]
    BnT_re = (Bn_re * ln_w[None, :]).T
    BnT_im = (Bn_im * ln_w[None, :]).T
    bc_re_v = Bn_re @ ln_b
    bc_im_v = Bn_im @ ln_b
    CT_re = C_re.T
    CT_imn = (-C_im).T
    DT = (D_m * ln_w[None, :]).T
    gbias_v = D_m @ ln_b
    WT = W_out.T.copy()
    WT[:, :D] *= 0.5
    b_a_v = 0.5 * b_out[:D]
    b_gh_v = 0.5 * b_out[D:]

    def cols(v, ntiles):
        return np.ascontiguousarray(np.asarray(v, np.float32).reshape(ntiles, P).T)

    return {
        "bt_re": _pack_kpm(BnT_re, KD, S).astype(NP16),
        "bt_im": _pack_kpm(BnT_im, KD, S).astype(NP16),
        "ct_re": _pack_kpm(CT_re, KS, D).astype(NP16),
        "ct_imn": _pack_kpm(CT_imn, KS, D).astype(NP16),
        "dt_w": _pack_kpm(DT, KD, D).astype(NP16),
        "wt": _pack_kpm(WT, KD, DFF).astype(NP16),
        "cosT": np.ascontiguousarray(
            cos_t.reshape(KS, P, L).transpose(1, 0, 2)
        ).astype(NP16),
        "sinT": np.ascontiguousarray(
            sin_t.reshape(KS, P, L).transpose(1, 0, 2)
        ).astype(NP16),
        "r_b": np.ascontiguousarray(
            np.broadcast_to(r.reshape(KS, P, 1), (KS, P, TC)).transpose(1, 0, 2)
        ).astype(np.float32),
        "bc_re": cols(bc_re_v, KS),
        "bc_im": cols(bc_im_v, KS),
        "gbias": cols(gbias_v, MD),
        "b_a": cols(b_a_v, MD),
        "b_gh": cols(b_gh_v, MD),
    }


def _make_in_maps(inputs):
    x = np.asarray(inputs["x"], np.float32)
    weights = _host_prepack(inputs)
    in_maps = []
    for b in range(B):
        xb = np.ascontiguousarray(x[b].T.reshape(KD, P, L).transpose(1, 0, 2))
        m = dict(weights)
        m["xT"] = xb
        m["xT16"] = xb.astype(NP16)
        in_maps.append(m)
    return in_maps


def kernel(**inputs):
    in_maps = _make_in_maps(inputs)
    with_bc = bool(np.any(np.asarray(inputs["ln_b"]) != 0))
    nc = _get_module(with_bc)
    res = bass_utils.run_bass_kernel_spmd(nc, in_maps, core_ids=list(range(N_CORES)))
    out = np.empty((B, L, D), np.float32)
    for b in range(B):
        ob = res.results[b]["outT"]
        out[b] = ob.transpose(1, 0, 2).reshape(D, L).T
    return out


# revision 16
# speedup vs baseline: 1.3388x; 1.2208x over previous
"""Trainium2 Bass kernel for the DWN block:
LayerNorm -> LRU (complex diagonal scan) -> GELU -> Linear(d,2d) -> GLU -> +x.

Strategy:
- Data-parallel: 1 batch element per NeuronCore (8 cores), SPMD NEFF.
- Transposed on-device layout [feature, time]: every matmul contracts the
  partition axis directly, and the LRU scan runs along the free axis.
- Complex scan decoupling: with lam = r*e^{i*theta} per state,
  u_t := e^{-i*theta*t} x_t obeys u_t = r*u_{t-1} + e^{-i*theta*t} b_t,
  i.e. two independent REAL first-order scans (re/im) per state ->
  hardware tensor_tensor_scan along the free axis. Twiddle factors
  cos/sin(theta*t) are precomputed on host in float64.
- LayerNorm stats for ALL time chunks are computed in a prologue via
  all-ones matmuls (result replicated across partitions); one batched
  Sqrt + fast-reciprocal gives rstd. ln_w/ln_b are folded into the
  downstream weights/biases on host.
- Matmul operands fp16 (fp32 PSUM accumulation, 1 cyc/row); scan decay
  r, GLU and residual fp32.
- ScalarE stays on the gelu_and_others table set (gelu/tanh/square/copy):
  sigmoid(g) is computed as 0.5 + 0.5*tanh(g/2) folded into the GLU math,
  so only ~2 ACT table loads happen for the whole kernel.
"""

import numpy as np

import concourse.bacc as bacc
import concourse.tile as tile
from concourse import mybir
from concourse import bass_utils

# ---- problem constants (hardcoded per contract) ----
B, L, D, S = 8, 2048, 512, 256
DFF = 2 * D
LN_EPS = 1e-5
N_CORES = 8

# ---- tiling ----
P = 128
TC = 512                 # time chunk
NCHUNK = L // TC         # 4
KD = D // P              # 4  k-tiles over d
KS = S // P              # 2  k-tiles over s
MD = D // P              # 4  m-tiles over d outputs

F32 = mybir.dt.float32
F16 = mybir.dt.float16
AOP = mybir.AluOpType
AF = mybir.ActivationFunctionType
NP16 = np.float16


def _pack_kpm(w, k_tiles, m):
    """[K, M] -> [128, k_tiles, M] host pack for lhsT storage (K = kt*128+p)."""
    K = k_tiles * P
    assert w.shape == (K, m)
    return np.ascontiguousarray(w.reshape(k_tiles, P, m).transpose(1, 0, 2))


def _build(nc, with_bc=False):
    f32 = F32
    f16 = F16

    xT = nc.dram_tensor("xT", [P, KD, L], f32, kind="ExternalInput")
    xT16 = nc.dram_tensor("xT16", [P, KD, L], f16, kind="ExternalInput")
    bt_re = nc.dram_tensor("bt_re", [P, KD, S], f16, kind="ExternalInput")
    bt_im = nc.dram_tensor("bt_im", [P, KD, S], f16, kind="ExternalInput")
    ct_re = nc.dram_tensor("ct_re", [P, KS, D], f16, kind="ExternalInput")
    ct_imn = nc.dram_tensor("ct_imn", [P, KS, D], f16, kind="ExternalInput")
    dt_w = nc.dram_tensor("dt_w", [P, KD, D], f16, kind="ExternalInput")
    wt = nc.dram_tensor("wt", [P, KD, DFF], f16, kind="ExternalInput")
    cosT = nc.dram_tensor("cosT", [P, KS, L], f16, kind="ExternalInput")
    sinT = nc.dram_tensor("sinT", [P, KS, L], f16, kind="ExternalInput")
    r_b = nc.dram_tensor("r_b", [P, KS, TC], f32, kind="ExternalInput")
    bc_re = nc.dram_tensor("bc_re", [P, KS], f32, kind="ExternalInput")
    bc_im = nc.dram_tensor("bc_im", [P, KS], f32, kind="ExternalInput")
    gbias = nc.dram_tensor("gbias", [P, MD], f32, kind="ExternalInput")
    b_a = nc.dram_tensor("b_a", [P, MD], f32, kind="ExternalInput")
    b_gh = nc.dram_tensor("b_gh", [P, MD], f32, kind="ExternalInput")
    outT = nc.dram_tensor("outT", [P, KD, L], f32, kind="ExternalOutput")

    with tile.TileContext(nc) as tc:
        with (
            tc.tile_pool(name="wpool", bufs=1) as wpool,
            tc.tile_pool(name="io", bufs=2) as io,
            tc.tile_pool(name="work", bufs=1) as work,
            tc.tile_pool(name="carry", bufs=2) as carry_pool,
            tc.tile_pool(name="psum", bufs=1, space="PSUM") as psum,
        ):
            # ---- resident weights/constants ----
            w_bt_re = wpool.tile([P, KD, S], f16)
            nc.sync.dma_start(w_bt_re[:], bt_re[:])
            w_bt_im = wpool.tile([P, KD, S], f16)
            nc.sync.dma_start(w_bt_im[:], bt_im[:])
            w_ct_re = wpool.tile([P, KS, D], f16)
            nc.sync.dma_start(w_ct_re[:], ct_re[:])
            w_ct_imn = wpool.tile([P, KS, D], f16)
            nc.sync.dma_start(w_ct_imn[:], ct_imn[:])
            w_dt = wpool.tile([P, KD, D], f16)
            nc.sync.dma_start(w_dt[:], dt_w[:])
            w_wt = wpool.tile([P, KD, DFF], f16)
            nc.sync.dma_start(w_wt[:], wt[:])
            w_r = wpool.tile([P, KS, TC], f32)
            nc.sync.dma_start(w_r[:], r_b[:])
            w_bc_re = wpool.tile([P, KS], f32)
            nc.sync.dma_start(w_bc_re[:], bc_re[:])
            w_bc_im = wpool.tile([P, KS], f32)
            nc.sync.dma_start(w_bc_im[:], bc_im[:])
            w_gbias = wpool.tile([P, MD], f32)
            nc.sync.dma_start(w_gbias[:], gbias[:])
            w_ba = wpool.tile([P, MD], f32)
            nc.sync.dma_start(w_ba[:], b_a[:])
            w_bgh = wpool.tile([P, MD], f32)
            nc.sync.dma_start(w_bgh[:], b_gh[:])
            ones = wpool.tile([P, P], f16)
            nc.vector.memset(ones, 1.0)
            w_eps = wpool.tile([P, 1], f32)
            nc.vector.memset(w_eps, LN_EPS)

            # ---- phase 0: LN stats for all chunks ----
            # x16 for the whole sequence stays resident (feeds xc later).
            x16_sb = wpool.tile([P, KD, L], f16)
            nc.sync.dma_start(x16_sb[:], xT16[:])
            mu16_all = wpool.tile([P, NCHUNK, TC], f16)
            var_all = wpool.tile([P, NCHUNK, TC], f32)
            for ck in range(NCHUNK):
                t0 = ck * TC
                x2_sb = work.tile([P, KD, TC], f16, tag="x2", bufs=2)
                for kt in range(KD):
                    nc.scalar.activation(
                        x2_sb[:, kt, :], x16_sb[:, kt, t0 : t0 + TC], AF.Square
                    )
                mu_ps = psum.tile([P, TC], f32, tag="pj", bufs=4, name=f"mu{ck}")
                msq_ps = psum.tile([P, TC], f32, tag="pj", bufs=4, name=f"msq{ck}")
                for kt in range(KD):
                    nc.tensor.matmul(
                        mu_ps[:], lhsT=ones[:], rhs=x16_sb[:, kt, t0 : t0 + TC],
                        start=(kt == 0), stop=(kt == KD - 1),
                    )
                for kt in range(KD):
                    nc.tensor.matmul(
                        msq_ps[:], lhsT=ones[:], rhs=x2_sb[:, kt, :],
                        start=(kt == 0), stop=(kt == KD - 1),
                    )
                # mu' (fp16, for xc) and var = msq/D - mu'^2
                nc.scalar.activation(
                    mu16_all[:, ck, :], mu_ps[:], AF.Copy, scale=1.0 / D
                )
                mu2 = work.tile([P, TC], f32, tag="mu2")
                nc.scalar.activation(mu2[:], mu_ps[:], AF.Square, scale=1.0 / D)
                nc.vector.scalar_tensor_tensor(
                    var_all[:, ck, :], msq_ps[:], 1.0 / D, mu2[:],
                    op0=AOP.mult, op1=AOP.subtract,
                )
            # sigma = sqrt(var + eps) over all chunks at once, then
            # rstd = 1/sigma via the fast custom-DVE reciprocal (~18 bits).
            sigma_all = work.tile([P, NCHUNK * TC], f32, tag="sigma", bufs=1)
            nc.scalar.activation(
                sigma_all[:], var_all.rearrange("p c t -> p (c t)"),
                AF.Sqrt, bias=w_eps[:],
            )
            rstd32 = work.tile([P, NCHUNK * TC], f32, tag="rstd32", bufs=1)
            nc.vector.reciprocal_approx_fast(rstd32[:], sigma_all[:])
            rstd16_all = wpool.tile([P, NCHUNK, TC], f16)
            nc.scalar.activation(
                rstd16_all.rearrange("p c t -> p (c t)"), rstd32[:], AF.Copy
            )
            # xhat for the whole sequence (feeds Bu and D matmuls directly)
            xhat_all = wpool.tile([P, KD, L], f16)
            for ck in range(NCHUNK):
                t0 = ck * TC
                for kt in range(KD):
                    xc = work.tile([P, TC], f16, tag="xc", bufs=3)
                    nc.vector.tensor_sub(
                        xc[:], x16_sb[:, kt, t0 : t0 + TC], mu16_all[:, ck, :]
                    )
                    nc.vector.tensor_mul(
                        xhat_all[:, kt, t0 : t0 + TC], xc[:],
                        rstd16_all[:, ck, :],
                    )

            # ---- main loop over time chunks ----
            u_prev = None
            for ck in range(NCHUNK):
                t0 = ck * TC

                x_sb = io.tile([P, KD, TC], f32, tag="x")
                nc.sync.dma_start(x_sb[:], xT[:, :, t0 : t0 + TC])
                cos_sb = io.tile([P, KS, TC], f16, tag="cos")
                nc.sync.dma_start(cos_sb[:], cosT[:, :, t0 : t0 + TC])
                sin_sb = io.tile([P, KS, TC], f16, tag="sin")
                nc.sync.dma_start(sin_sb[:], sinT[:, :, t0 : t0 + TC])
                cos_g = io.tile([P, TC], f16, tag="cosg")
                nc.sync.dma_start(cos_g[:], cosT[:, 1, t0 : t0 + TC])
                sin_g = io.tile([P, TC], f16, tag="sing")
                nc.sync.dma_start(sin_g[:], sinT[:, 1, t0 : t0 + TC])

                # ---- Bu matmuls -> psum (4 banks: [re/im] x [s-tile]) ----
                ps_bu = [
                    [
                        psum.tile([P, TC], f32, tag="bu", bufs=2, name=f"bu{c}{st}")
                        for st in range(KS)
                    ]
                    for c in range(2)
                ]
                for st in range(KS):
                    for comp, w_bt in ((0, w_bt_re), (1, w_bt_im)):
                        for kt in range(KD):
                            nc.tensor.matmul(
                                ps_bu[comp][st][:],
                                lhsT=w_bt[:, kt, st * P : (st + 1) * P],
                                rhs=xhat_all[:, kt, t0 : t0 + TC],
                                start=(kt == 0),
                                stop=(kt == KD - 1),
                            )

                # ---- evac Bu (+ state bias bc = B_norm @ ln_b) to fp16 ----
                bu_re = work.tile([P, KS, TC], f16, tag="bu_re")
                bu_im = work.tile([P, KS, TC], f16, tag="bu_im")
                for st in range(KS):
                    if with_bc:
                        nc.vector.tensor_scalar_add(
                            bu_re[:, st, :], ps_bu[0][st][:],
                            w_bc_re[:, st : st + 1],
                        )
                        nc.vector.tensor_scalar_add(
                            bu_im[:, st, :], ps_bu[1][st][:],
                            w_bc_im[:, st : st + 1],
                        )
                    else:
                        nc.scalar.activation(
                            bu_re[:, st, :], ps_bu[0][st][:], AF.Copy)
                        nc.scalar.activation(
                            bu_im[:, st, :], ps_bu[1][st][:], AF.Copy)

                # ---- twiddle: c = e^{-i theta t} * Bu ----
                c_re = work.tile([P, KS, TC], f16, tag="c_re")
                c_im = work.tile([P, KS, TC], f16, tag="c_im")
                tw1 = work.tile([P, KS, TC], f16, tag="tw1")
                tw2 = work.tile([P, KS, TC], f16, tag="tw2")
                for st in range(KS):
                    nc.vector.tensor_mul(tw1[:, st, :], cos_sb[:, st, :], bu_re[:, st, :])
                    nc.vector.tensor_mul(tw2[:, st, :], sin_sb[:, st, :], bu_im[:, st, :])
                    nc.vector.tensor_add(c_re[:, st, :], tw1[:, st, :], tw2[:, st, :])
                    nc.vector.tensor_mul(tw1[:, st, :], cos_sb[:, st, :], bu_im[:, st, :])
                    nc.vector.tensor_mul(tw2[:, st, :], sin_sb[:, st, :], bu_re[:, st, :])
                    nc.vector.tensor_sub(c_im[:, st, :], tw1[:, st, :], tw2[:, st, :])

                # ---- scans: u_t = r*u_{t-1} + c_t (re/im per s-tile) ----
                u = carry_pool.tile([P, 2, KS, TC], f16, tag="u")
                for comp, c_t in ((0, c_re), (1, c_im)):
                    for st in range(KS):
                        init = (
                            0.0 if u_prev is None
                            else u_prev[:, comp, st, TC - 1 : TC]
                        )
                        nc.vector.tensor_tensor_scan(
                            u[:, comp, st, :],
                            w_r[:, st, :],
                            c_t[:, st, :],
                            init,
                            op0=AOP.mult,
                            op1=AOP.add,
                        )
                u_prev = u

                # ---- untwiddle: xs = e^{+i theta t} u ----
                # s-tile 0 on VectorE, s-tile 1 on GpSimd (load balance)
                xs_re = work.tile([P, KS, TC], f16, tag="xs_re")
                xs_im = work.tile([P, KS, TC], f16, tag="xs_im")
                tg1 = work.tile([P, TC], f16, tag="tg1")
                tg2 = work.tile([P, TC], f16, tag="tg2")
                for st in range(KS):
                    eng = nc.vector if st == 0 else nc.gpsimd
                    cs = cos_sb[:, st, :] if st == 0 else cos_g[:]
                    sn = sin_sb[:, st, :] if st == 0 else sin_g[:]
                    t1 = tw1[:, st, :] if st == 0 else tg1[:]
                    t2 = tw2[:, st, :] if st == 0 else tg2[:]
                    eng.tensor_mul(t1, cs, u[:, 0, st, :])
                    eng.tensor_mul(t2, sn, u[:, 1, st, :])
                    eng.tensor_sub(xs_re[:, st, :], t1, t2)
                    eng.tensor_mul(t1, sn, u[:, 0, st, :])
                    eng.tensor_mul(t2, cs, u[:, 1, st, :])
                    eng.tensor_add(xs_im[:, st, :], t1, t2)

                # ---- y = C_re@xs_re + (-C_im)@xs_im + (D.w)@xhat -> gelu ----
                h_sb = work.tile([P, MD, TC], f16, tag="h")
                for mt in range(MD):
                    ps_y = psum.tile([P, TC], f32, tag="y", bufs=2, name=f"y{mt}")
                    for kt in range(KD):
                        nc.tensor.matmul(
                            ps_y[:],
                            lhsT=w_dt[:, kt, mt * P : (mt + 1) * P],
                            rhs=xhat_all[:, kt, t0 : t0 + TC],
                            start=(kt == 0), stop=False,
                        )
                    for st in range(KS):
                        nc.tensor.matmul(
                            ps_y[:],
                            lhsT=w_ct_re[:, st, mt * P : (mt + 1) * P],
                            rhs=xs_re[:, st, :],
                            start=False, stop=False,
                        )
                    for st in range(KS):
                        nc.tensor.matmul(
                            ps_y[:],
                            lhsT=w_ct_imn[:, st, mt * P : (mt + 1) * P],
                            rhs=xs_im[:, st, :],
                            start=False, stop=(st == KS - 1),
                        )
                    nc.scalar.activation(
                        h_sb[:, mt, :], ps_y[:], AF.Gelu,
                        bias=w_gbias[:, mt : mt + 1],
                    )

                # ---- proj = W.h ; GLU via tanh ; residual ----
                # sigmoid(g+bg) = 0.5 + 0.5*tanh((g+bg)/2)
                # out = (a+ba)*sig + x = 0.5*[(a+ba) + (a+ba)*t] + x
                out_sb = io.tile([P, KD, TC], f32, tag="out")
                for mt in range(MD):
                    th = work.tile([P, TC], f32, tag="th", bufs=2)
                    g1 = work.tile([P, TC], f32, tag="g1", bufs=2)
                    g2 = work.tile([P, TC], f32, tag="g2", bufs=2)
                    ps_pa = psum.tile([P, TC], f32, tag="pj", bufs=4, name=f"pa{mt}")
                    ps_pg = psum.tile([P, TC], f32, tag="pj", bufs=4, name=f"pg{mt}")
                    for kt in range(KD):
                        nc.tensor.matmul(
                            ps_pa[:],
                            lhsT=w_wt[:, kt, mt * P : (mt + 1) * P],
                            rhs=h_sb[:, kt, :],
                            start=(kt == 0), stop=(kt == KD - 1),
                        )
                    for kt in range(KD):
                        nc.tensor.matmul(
                            ps_pg[:],
                            lhsT=w_wt[:, kt, D + mt * P : D + (mt + 1) * P],
                            rhs=h_sb[:, kt, :],
                            start=(kt == 0), stop=(kt == KD - 1),
                        )
                    nc.scalar.activation(
                        th[:], ps_pg[:], AF.Tanh,
                        bias=w_bgh[:, mt : mt + 1], scale=0.5,
                    )
                    nc.vector.scalar_tensor_tensor(
                        g1[:], ps_pa[:], w_ba[:, mt : mt + 1],
                        th[:], op0=AOP.add, op1=AOP.mult,
                    )
                    nc.vector.scalar_tensor_tensor(
                        g2[:], ps_pa[:], w_ba[:, mt : mt + 1],
                        g1[:], op0=AOP.add, op1=AOP.add,
                    )
                    nc.gpsimd.tensor_add(
                        out_sb[:, mt, :], g2[:], x_sb[:, mt, :]
                    )

                nc.sync.dma_start(outT[:, :, t0 : t0 + TC], out_sb[:])

    nc.compile()
    return nc


_NC_CACHE = {}


def _get_module(with_bc=False):
    if with_bc not in _NC_CACHE:
        nc = bacc.Bacc("TRN2", target_bir_lowering=False, debug=False)
        _NC_CACHE[with_bc] = _build(nc, with_bc=with_bc)
    return _NC_CACHE[with_bc]


def _host_prepack(inputs):
    ln_w = np.asarray(inputs["ln_w"], np.float64)
    ln_b = np.asarray(inputs["ln_b"], np.float64)
    nu_log = np.asarray(inputs["nu_log"], np.float64)
    theta_log = np.asarray(inputs["theta_log"], np.float64)
    gamma_log = np.asarray(inputs["gamma_log"], np.float64)
    B_re = np.asarray(inputs["B_re"], np.float64)
    B_im = np.asarray(inputs["B_im"], np.float64)
    C_re = np.asarray(inputs["C_re"], np.float64)
    C_im = np.asarray(inputs["C_im"], np.float64)
    D_m = np.asarray(inputs["D"], np.float64)
    W_out = np.asarray(inputs["W_out"], np.float64)
    b_out = np.asarray(inputs["b_out"], np.float64)

    r = np.exp(-np.exp(nu_log))
    theta = np.exp(theta_log)
    g = np.exp(gamma_log)
    ang = theta[:, None] * np.arange(L, dtype=np.float64)[None, :]
    cos_t = np.cos(ang)
    sin_t = np.sin(ang)

    Bn_re = B_re * g[:, None]
    Bn_im = B_im * g[:, None]
    BnT_re = (Bn_re * ln_w[None, :]).T
    BnT_im = (Bn_im * ln_w[None, :]).T
    bc_re_v = Bn_re @ ln_b
    bc_im_v = Bn_im @ ln_b
    CT_re = C_re.T
    CT_imn = (-C_im).T
    DT = (D_m * ln_w[None, :]).T
    gbias_v = D_m @ ln_b
    WT = W_out.T.copy()
    WT[:, :D] *= 0.5
    b_a_v = 0.5 * b_out[:D]
    b_gh_v = 0.5 * b_out[D:]

    def cols(v, ntiles):
        return np.ascontiguousarray(np.asarray(v, np.float32).reshape(ntiles, P).T)

    return {
        "bt_re": _pack_kpm(BnT_re, KD, S).astype(NP16),
        "bt_im": _pack_kpm(BnT_im, KD, S).astype(NP16),
        "ct_re": _pack_kpm(CT_re, KS, D).astype(NP16),
        "ct_imn": _pack_kpm(CT_imn, KS, D).astype(NP16),
        "dt_w": _pack_kpm(DT, KD, D).astype(NP16),
        "wt": _pack_kpm(WT, KD, DFF).astype(NP16),
        "cosT": np.ascontiguousarray(
            cos_t.reshape(KS, P, L).transpose(1, 0, 2)
        ).astype(NP16),
        "sinT": np.ascontiguousarray(
            sin_t.reshape(KS, P, L).transpose(1, 0, 2)
        ).astype(NP16),
        "r_b": np.ascontiguousarray(
            np.broadcast_to(r.reshape(KS, P, 1), (KS, P, TC)).transpose(1, 0, 2)
        ).astype(np.float32),
        "bc_re": cols(bc_re_v, KS),
        "bc_im": cols(bc_im_v, KS),
        "gbias": cols(gbias_v, MD),
        "b_a": cols(b_a_v, MD),
        "b_gh": cols(b_gh_v, MD),
    }


def _make_in_maps(inputs):
    x = np.asarray(inputs["x"], np.float32)
    weights = _host_prepack(inputs)
    in_maps = []
    for b in range(B):
        xb = np.ascontiguousarray(x[b].T.reshape(KD, P, L).transpose(1, 0, 2))
        m = dict(weights)
        m["xT"] = xb
        m["xT16"] = xb.astype(NP16)
        in_maps.append(m)
    return in_maps


def kernel(**inputs):
    in_maps = _make_in_maps(inputs)
    with_bc = bool(np.any(np.asarray(inputs["ln_b"]) != 0))
    nc = _get_module(with_bc)
    res = bass_utils.run_bass_kernel_spmd(nc, in_maps, core_ids=list(range(N_CORES)))
    out = np.empty((B, L, D), np.float32)
    for b in range(B):
        ob = res.results[b]["outT"]
        out[b] = ob.transpose(1, 0, 2).reshape(D, L).T
    return out


# revision 17
# speedup vs baseline: 1.4768x; 1.1031x over previous
"""Trainium2 Bass kernel for the DWN block:
LayerNorm -> LRU (complex diagonal scan) -> GELU -> Linear(d,2d) -> GLU -> +x.

Strategy:
- Data-parallel: 1 batch element per NeuronCore (8 cores), SPMD NEFF.
- Transposed on-device layout [feature, time]: every matmul contracts the
  partition axis directly, and the LRU scan runs along the free axis.
- Complex scan decoupling: with lam = r*e^{i*theta} per state,
  u_t := e^{-i*theta*t} x_t obeys u_t = r*u_{t-1} + e^{-i*theta*t} b_t,
  i.e. two independent REAL first-order scans (re/im) per state ->
  hardware tensor_tensor_scan along the free axis. Twiddle factors
  cos/sin(theta*t) are precomputed on host in float64.
- LayerNorm stats for ALL time chunks are computed in a prologue via
  all-ones matmuls (result replicated across partitions); one batched
  Sqrt + fast-reciprocal gives rstd. ln_w/ln_b are folded into the
  downstream weights/biases on host.
- Matmul operands fp16 (fp32 PSUM accumulation, 1 cyc/row); scan decay
  r, GLU and residual fp32.
- ScalarE stays on the gelu_and_others table set (gelu/tanh/square/copy):
  sigmoid(g) is computed as 0.5 + 0.5*tanh(g/2) folded into the GLU math,
  so only ~2 ACT table loads happen for the whole kernel.
"""

import numpy as np

import concourse.bacc as bacc
import concourse.tile as tile
from concourse import mybir
from concourse import bass_utils

# ---- problem constants (hardcoded per contract) ----
B, L, D, S = 8, 2048, 512, 256
DFF = 2 * D
LN_EPS = 1e-5
N_CORES = 8

# ---- tiling ----
P = 128
TC = 512                 # time chunk
NCHUNK = L // TC         # 4
KD = D // P              # 4  k-tiles over d
KS = S // P              # 2  k-tiles over s
MD = D // P              # 4  m-tiles over d outputs

F32 = mybir.dt.float32
F16 = mybir.dt.float16
AOP = mybir.AluOpType
AF = mybir.ActivationFunctionType
NP16 = np.float16


def _pack_kpm(w, k_tiles, m):
    """[K, M] -> [128, k_tiles, M] host pack for lhsT storage (K = kt*128+p)."""
    K = k_tiles * P
    assert w.shape == (K, m)
    return np.ascontiguousarray(w.reshape(k_tiles, P, m).transpose(1, 0, 2))


def _build(nc, with_bc=False):
    f32 = F32
    f16 = F16

    xT = nc.dram_tensor("xT", [P, KD, L], f32, kind="ExternalInput")
    xT16 = nc.dram_tensor("xT16", [P, KD, L], f16, kind="ExternalInput")
    bt_re = nc.dram_tensor("bt_re", [P, KD, S], f16, kind="ExternalInput")
    bt_im = nc.dram_tensor("bt_im", [P, KD, S], f16, kind="ExternalInput")
    ct_re = nc.dram_tensor("ct_re", [P, KS, D], f16, kind="ExternalInput")
    ct_imn = nc.dram_tensor("ct_imn", [P, KS, D], f16, kind="ExternalInput")
    dt_w = nc.dram_tensor("dt_w", [P, KD, D], f16, kind="ExternalInput")
    wt = nc.dram_tensor("wt", [P, KD, DFF], f16, kind="ExternalInput")
    cosT = nc.dram_tensor("cosT", [P, KS, L], f16, kind="ExternalInput")
    sinT = nc.dram_tensor("sinT", [P, KS, L], f16, kind="ExternalInput")
    r_b = nc.dram_tensor("r_b", [P, KS, TC], f32, kind="ExternalInput")
    bc_re = nc.dram_tensor("bc_re", [P, KS], f32, kind="ExternalInput")
    bc_im = nc.dram_tensor("bc_im", [P, KS], f32, kind="ExternalInput")
    gbias = nc.dram_tensor("gbias", [P, MD], f32, kind="ExternalInput")
    b_a = nc.dram_tensor("b_a", [P, MD], f32, kind="ExternalInput")
    b_gh = nc.dram_tensor("b_gh", [P, MD], f32, kind="ExternalInput")
    outT = nc.dram_tensor("outT", [P, KD, L], f32, kind="ExternalOutput")

    with tile.TileContext(nc) as tc:
        with (
            tc.tile_pool(name="wpool", bufs=1) as wpool,
            tc.tile_pool(name="io", bufs=2) as io,
            tc.tile_pool(name="work", bufs=1) as work,
            tc.tile_pool(name="carry", bufs=2) as carry_pool,
            tc.tile_pool(name="psum", bufs=1, space="PSUM") as psum,
        ):
            # ---- resident weights/constants ----
            w_bt_re = wpool.tile([P, KD, S], f16)
            nc.sync.dma_start(w_bt_re[:], bt_re[:])
            w_bt_im = wpool.tile([P, KD, S], f16)
            nc.sync.dma_start(w_bt_im[:], bt_im[:])
            w_ct_re = wpool.tile([P, KS, D], f16)
            nc.sync.dma_start(w_ct_re[:], ct_re[:])
            w_ct_imn = wpool.tile([P, KS, D], f16)
            nc.sync.dma_start(w_ct_imn[:], ct_imn[:])
            w_dt = wpool.tile([P, KD, D], f16)
            nc.sync.dma_start(w_dt[:], dt_w[:])
            w_wt = wpool.tile([P, KD, DFF], f16)
            nc.sync.dma_start(w_wt[:], wt[:])
            w_r = wpool.tile([P, KS, TC], f32)
            nc.sync.dma_start(w_r[:], r_b[:])
            w_bc_re = wpool.tile([P, KS], f32)
            nc.sync.dma_start(w_bc_re[:], bc_re[:])
            w_bc_im = wpool.tile([P, KS], f32)
            nc.sync.dma_start(w_bc_im[:], bc_im[:])
            w_gbias = wpool.tile([P, MD], f32)
            nc.sync.dma_start(w_gbias[:], gbias[:])
            w_ba = wpool.tile([P, MD], f32)
            nc.sync.dma_start(w_ba[:], b_a[:])
            w_bgh = wpool.tile([P, MD], f32)
            nc.sync.dma_start(w_bgh[:], b_gh[:])
            ones = wpool.tile([P, P], f16)
            nc.vector.memset(ones, 1.0)
            w_eps = wpool.tile([P, 1], f32)
            nc.vector.memset(w_eps, LN_EPS)

            # ---- phase 0: LN stats for all chunks ----
            # x16 for the whole sequence stays resident (feeds xc later).
            x16_sb = wpool.tile([P, KD, L], f16)
            nc.sync.dma_start(x16_sb[:], xT16[:])
            mu16_all = wpool.tile([P, NCHUNK, TC], f16)
            var_all = wpool.tile([P, NCHUNK, TC], f32)
            for ck in range(NCHUNK):
                t0 = ck * TC
                x2_sb = work.tile([P, KD, TC], f16, tag="x2", bufs=2)
                for kt in range(KD):
                    nc.scalar.activation(
                        x2_sb[:, kt, :], x16_sb[:, kt, t0 : t0 + TC], AF.Square
                    )
                mu_ps = psum.tile([P, TC], f32, tag="pj", bufs=4, name=f"mu{ck}")
                msq_ps = psum.tile([P, TC], f32, tag="pj", bufs=4, name=f"msq{ck}")
                for kt in range(KD):
                    nc.tensor.matmul(
                        mu_ps[:], lhsT=ones[:], rhs=x16_sb[:, kt, t0 : t0 + TC],
                        start=(kt == 0), stop=(kt == KD - 1),
                    )
                for kt in range(KD):
                    nc.tensor.matmul(
                        msq_ps[:], lhsT=ones[:], rhs=x2_sb[:, kt, :],
                        start=(kt == 0), stop=(kt == KD - 1),
                    )
                # mu' (fp16, for xc) and var = msq/D - mu'^2
                nc.scalar.activation(
                    mu16_all[:, ck, :], mu_ps[:], AF.Copy, scale=1.0 / D
                )
                mu2 = work.tile([P, TC], f32, tag="mu2")
                nc.scalar.activation(mu2[:], mu_ps[:], AF.Square, scale=1.0 / D)
                nc.vector.scalar_tensor_tensor(
                    var_all[:, ck, :], msq_ps[:], 1.0 / D, mu2[:],
                    op0=AOP.mult, op1=AOP.subtract,
                )
            # sigma = sqrt(var + eps) over all chunks at once, then
            # rstd = 1/sigma via the fast custom-DVE reciprocal (~18 bits).
            sigma_all = work.tile([P, NCHUNK * TC], f32, tag="sigma", bufs=1)
            nc.scalar.activation(
                sigma_all[:], var_all.rearrange("p c t -> p (c t)"),
                AF.Sqrt, bias=w_eps[:],
            )
            rstd32 = work.tile([P, NCHUNK * TC], f32, tag="rstd32", bufs=1)
            nc.vector.reciprocal_approx_fast(rstd32[:], sigma_all[:])
            rstd16_all = wpool.tile([P, NCHUNK, TC], f16)
            nc.scalar.activation(
                rstd16_all.rearrange("p c t -> p (c t)"), rstd32[:], AF.Copy
            )
            # xhat for the whole sequence (feeds Bu and D matmuls directly)
            xhat_all = wpool.tile([P, KD, L], f16)
            for ck in range(NCHUNK):
                t0 = ck * TC
                for kt in range(KD):
                    xc = work.tile([P, TC], f16, tag="xc", bufs=3)
                    nc.vector.tensor_sub(
                        xc[:], x16_sb[:, kt, t0 : t0 + TC], mu16_all[:, ck, :]
                    )
                    nc.vector.tensor_mul(
                        xhat_all[:, kt, t0 : t0 + TC], xc[:],
                        rstd16_all[:, ck, :],
                    )

            # ---- main loop over time chunks ----
            u_prev = None
            for ck in range(NCHUNK):
                t0 = ck * TC

                x_sb = io.tile([P, KD, TC], f32, tag="x")
                nc.sync.dma_start(x_sb[:], xT[:, :, t0 : t0 + TC])
                cos_sb = io.tile([P, KS, TC], f16, tag="cos")
                nc.sync.dma_start(cos_sb[:], cosT[:, :, t0 : t0 + TC])
                sin_sb = io.tile([P, KS, TC], f16, tag="sin")
                nc.sync.dma_start(sin_sb[:], sinT[:, :, t0 : t0 + TC])

                # ---- Bu matmuls -> psum (4 banks: [re/im] x [s-tile]) ----
                ps_bu = [
                    [
                        psum.tile([P, TC], f32, tag="bu", bufs=2, name=f"bu{c}{st}")
                        for st in range(KS)
                    ]
                    for c in range(2)
                ]
                for st in range(KS):
                    for comp, w_bt in ((0, w_bt_re), (1, w_bt_im)):
                        for kt in range(KD):
                            nc.tensor.matmul(
                                ps_bu[comp][st][:],
                                lhsT=w_bt[:, kt, st * P : (st + 1) * P],
                                rhs=xhat_all[:, kt, t0 : t0 + TC],
                                start=(kt == 0),
                                stop=(kt == KD - 1),
                            )

                # ---- evac Bu (+ state bias bc = B_norm @ ln_b) to fp16 ----
                bu_re = work.tile([P, KS, TC], f16, tag="bu_re")
                bu_im = work.tile([P, KS, TC], f16, tag="bu_im")
                for st in range(KS):
                    if with_bc:
                        nc.vector.tensor_scalar_add(
                            bu_re[:, st, :], ps_bu[0][st][:],
                            w_bc_re[:, st : st + 1],
                        )
                        nc.vector.tensor_scalar_add(
                            bu_im[:, st, :], ps_bu[1][st][:],
                            w_bc_im[:, st : st + 1],
                        )
                    else:
                        nc.scalar.activation(
                            bu_re[:, st, :], ps_bu[0][st][:], AF.Copy)
                        nc.scalar.activation(
                            bu_im[:, st, :], ps_bu[1][st][:], AF.Copy)

                # ---- twiddle: c = e^{-i theta t} * Bu ----
                c_re = work.tile([P, KS, TC], f16, tag="c_re")
                c_im = work.tile([P, KS, TC], f16, tag="c_im")
                tw1 = work.tile([P, KS, TC], f16, tag="tw1")
                tw2 = work.tile([P, KS, TC], f16, tag="tw2")
                for st in range(KS):
                    nc.vector.tensor_mul(tw1[:, st, :], cos_sb[:, st, :], bu_re[:, st, :])
                    nc.vector.tensor_mul(tw2[:, st, :], sin_sb[:, st, :], bu_im[:, st, :])
                    nc.vector.tensor_add(c_re[:, st, :], tw1[:, st, :], tw2[:, st, :])
                    nc.vector.tensor_mul(tw1[:, st, :], cos_sb[:, st, :], bu_im[:, st, :])
                    nc.vector.tensor_mul(tw2[:, st, :], sin_sb[:, st, :], bu_re[:, st, :])
                    nc.vector.tensor_sub(c_im[:, st, :], tw1[:, st, :], tw2[:, st, :])

                # ---- scans: u_t = r*u_{t-1} + c_t (re/im per s-tile) ----
                u = carry_pool.tile([P, 2, KS, TC], f16, tag="u")
                for comp, c_t in ((0, c_re), (1, c_im)):
                    for st in range(KS):
                        init = (
                            0.0 if u_prev is None
                            else u_prev[:, comp, st, TC - 1 : TC]
                        )
                        nc.vector.tensor_tensor_scan(
                            u[:, comp, st, :],
                            w_r[:, st, :],
                            c_t[:, st, :],
                            init,
                            op0=AOP.mult,
                            op1=AOP.add,
                        )
                u_prev = u

                # ---- untwiddle: xs = e^{+i theta t} u ----
                # s-tile 0 on VectorE, s-tile 1 on GpSimd (load balance)
                xs_re = work.tile([P, KS, TC], f16, tag="xs_re")
                xs_im = work.tile([P, KS, TC], f16, tag="xs_im")
                for st in range(KS):
                    nc.vector.tensor_mul(tw1[:, st, :], cos_sb[:, st, :], u[:, 0, st, :])
                    nc.vector.tensor_mul(tw2[:, st, :], sin_sb[:, st, :], u[:, 1, st, :])
                    nc.vector.tensor_sub(xs_re[:, st, :], tw1[:, st, :], tw2[:, st, :])
                    nc.vector.tensor_mul(tw1[:, st, :], sin_sb[:, st, :], u[:, 0, st, :])
                    nc.vector.tensor_mul(tw2[:, st, :], cos_sb[:, st, :], u[:, 1, st, :])
                    nc.vector.tensor_add(xs_im[:, st, :], tw1[:, st, :], tw2[:, st, :])

                # ---- y = C_re@xs_re + (-C_im)@xs_im + (D.w)@xhat -> gelu ----
                h_sb = work.tile([P, MD, TC], f16, tag="h")
                for mt in range(MD):
                    ps_y = psum.tile([P, TC], f32, tag="y", bufs=2, name=f"y{mt}")
                    for kt in range(KD):
                        nc.tensor.matmul(
                            ps_y[:],
                            lhsT=w_dt[:, kt, mt * P : (mt + 1) * P],
                            rhs=xhat_all[:, kt, t0 : t0 + TC],
                            start=(kt == 0), stop=False,
                        )
                    for st in range(KS):
                        nc.tensor.matmul(
                            ps_y[:],
                            lhsT=w_ct_re[:, st, mt * P : (mt + 1) * P],
                            rhs=xs_re[:, st, :],
                            start=False, stop=False,
                        )
                    for st in range(KS):
                        nc.tensor.matmul(
                            ps_y[:],
                            lhsT=w_ct_imn[:, st, mt * P : (mt + 1) * P],
                            rhs=xs_im[:, st, :],
                            start=False, stop=(st == KS - 1),
                        )
                    nc.scalar.activation(
                        h_sb[:, mt, :], ps_y[:], AF.Gelu,
                        bias=w_gbias[:, mt : mt + 1],
                    )

                # ---- proj = W.h ; GLU via tanh ; residual ----
                # sigmoid(g+bg) = 0.5 + 0.5*tanh((g+bg)/2)
                # out = (a+ba)*sig + x = 0.5*[(a+ba) + (a+ba)*t] + x
                out_sb = io.tile([P, KD, TC], f32, tag="out")
                for mt in range(MD):
                    th = work.tile([P, TC], f16, tag="th", bufs=2)
                    w16 = work.tile([P, TC], f16, tag="w16", bufs=2)
                    q = work.tile([P, TC], f32, tag="q", bufs=2)
                    ps_pa = psum.tile([P, TC], f32, tag="pj", bufs=4, name=f"pa{mt}")
                    ps_pg = psum.tile([P, TC], f32, tag="pj", bufs=4, name=f"pg{mt}")
                    for kt in range(KD):
                        nc.tensor.matmul(
                            ps_pa[:],
                            lhsT=w_wt[:, kt, mt * P : (mt + 1) * P],
                            rhs=h_sb[:, kt, :],
                            start=(kt == 0), stop=(kt == KD - 1),
                        )
                    for kt in range(KD):
                        nc.tensor.matmul(
                            ps_pg[:],
                            lhsT=w_wt[:, kt, D + mt * P : D + (mt + 1) * P],
                            rhs=h_sb[:, kt, :],
                            start=(kt == 0), stop=(kt == KD - 1),
                        )
                    nc.scalar.activation(
                        th[:], ps_pg[:], AF.Tanh,
                        bias=w_bgh[:, mt : mt + 1], scale=0.5,
                    )
                    nc.vector.tensor_scalar_add(w16[:], th[:], 1.0)
                    nc.vector.scalar_tensor_tensor(
                        q[:], ps_pa[:], w_ba[:, mt : mt + 1],
                        w16[:], op0=AOP.add, op1=AOP.mult,
                    )
                    nc.vector.tensor_add(
                        out_sb[:, mt, :], q[:], x_sb[:, mt, :]
                    )

                nc.sync.dma_start(outT[:, :, t0 : t0 + TC], out_sb[:])

    nc.compile()
    return nc


_NC_CACHE = {}


def _get_module(with_bc=False):
    if with_bc not in _NC_CACHE:
        nc = bacc.Bacc("TRN2", target_bir_lowering=False, debug=False)
        _NC_CACHE[with_bc] = _build(nc, with_bc=with_bc)
    return _NC_CACHE[with_bc]


def _host_prepack(inputs):
    ln_w = np.asarray(inputs["ln_w"], np.float64)
    ln_b = np.asarray(inputs["ln_b"], np.float64)
    nu_log = np.asarray(inputs["nu_log"], np.float64)
    theta_log = np.asarray(inputs["theta_log"], np.float64)
    gamma_log = np.asarray(inputs["gamma_log"], np.float64)
    B_re = np.asarray(inputs["B_re"], np.float64)
    B_im = np.asarray(inputs["B_im"], np.float64)
    C_re = np.asarray(inputs["C_re"], np.float64)
    C_im = np.asarray(inputs["C_im"], np.float64)
    D_m = np.asarray(inputs["D"], np.float64)
    W_out = np.asarray(inputs["W_out"], np.float64)
    b_out = np.asarray(inputs["b_out"], np.float64)

    r = np.exp(-np.exp(nu_log))
    theta = np.exp(theta_log)
    g = np.exp(gamma_log)
    ang = theta[:, None] * np.arange(L, dtype=np.float64)[None, :]
    cos_t = np.cos(ang)
    sin_t = np.sin(ang)

    Bn_re = B_re * g[:, None]
    Bn_im = B_im * g[:, None]
    BnT_re = (Bn_re * ln_w[None, :]).T
    BnT_im = (Bn_im * ln_w[None, :]).T
    bc_re_v = Bn_re @ ln_b
    bc_im_v = Bn_im @ ln_b
    CT_re = C_re.T
    CT_imn = (-C_im).T
    DT = (D_m * ln_w[None, :]).T
    gbias_v = D_m @ ln_b
    WT = W_out.T.copy()
    WT[:, :D] *= 0.5
    b_a_v = 0.5 * b_out[:D]
    b_gh_v = 0.5 * b_out[D:]

    def cols(v, ntiles):
        return np.ascontiguousarray(np.asarray(v, np.float32).reshape(ntiles, P).T)

    return {
        "bt_re": _pack_kpm(BnT_re, KD, S).astype(NP16),
        "bt_im": _pack_kpm(BnT_im, KD, S).astype(NP16),
        "ct_re": _pack_kpm(CT_re, KS, D).astype(NP16),
        "ct_imn": _pack_kpm(CT_imn, KS, D).astype(NP16),
        "dt_w": _pack_kpm(DT, KD, D).astype(NP16),
        "wt": _pack_kpm(WT, KD, DFF).astype(NP16),
        "cosT": np.ascontiguousarray(
            cos_t.reshape(KS, P, L).transpose(1, 0, 2)
        ).astype(NP16),
        "sinT": np.ascontiguousarray(
            sin_t.reshape(KS, P, L).transpose(1, 0, 2)
        ).astype(NP16),
        "r_b": np.ascontiguousarray(
            np.broadcast_to(r.reshape(KS, P, 1), (KS, P, TC)).transpose(1, 0, 2)
        ).astype(np.float32),
        "bc_re": cols(bc_re_v, KS),
        "bc_im": cols(bc_im_v, KS),
        "gbias": cols(gbias_v, MD),
        "b_a": cols(b_a_v, MD),
        "b_gh": cols(b_gh_v, MD),
    }


def _make_in_maps(inputs):
    x = np.asarray(inputs["x"], np.float32)
    weights = _host_prepack(inputs)
    in_maps = []
    for b in range(B):
        xb = np.ascontiguousarray(x[b].T.reshape(KD, P, L).transpose(1, 0, 2))
        m = dict(weights)
        m["xT"] = xb
        m["xT16"] = xb.astype(NP16)
        in_maps.append(m)
    return in_maps


def kernel(**inputs):
    in_maps = _make_in_maps(inputs)
    with_bc = bool(np.any(np.asarray(inputs["ln_b"]) != 0))
    nc = _get_module(with_bc)
    res = bass_utils.run_bass_kernel_spmd(nc, in_maps, core_ids=list(range(N_CORES)))
    out = np.empty((B, L, D), np.float32)
    for b in range(B):
        ob = res.results[b]["outT"]
        out[b] = ob.transpose(1, 0, 2).reshape(D, L).T
    return out


# revision 18
# speedup vs baseline: 1.7003x; 1.1514x over previous
"""Trainium2 Bass kernel for the DWN block:
LayerNorm -> LRU (complex diagonal scan) -> GELU -> Linear(d,2d) -> GLU -> +x.

Strategy:
- Data-parallel: 1 batch element per NeuronCore (8 cores), SPMD NEFF.
- Transposed on-device layout [feature, time]: every matmul contracts the
  partition axis directly, and the LRU scan runs along the free axis.
- Complex scan decoupling: with lam = r*e^{i*theta} per state,
  u_t := e^{-i*theta*t} x_t obeys u_t = r*u_{t-1} + e^{-i*theta*t} b_t,
  i.e. two independent REAL first-order scans (re/im) per state ->
  hardware tensor_tensor_scan along the free axis. Twiddle factors
  cos/sin(theta*t) are precomputed on host in float64.
- LayerNorm stats for ALL time chunks are computed in a prologue via
  all-ones matmuls (result replicated across partitions); one batched
  Sqrt + fast-reciprocal gives rstd. ln_w/ln_b are folded into the
  downstream weights/biases on host.
- Matmul operands fp16 (fp32 PSUM accumulation, 1 cyc/row); scan decay
  r, GLU and residual fp32.
- ScalarE stays on the gelu_and_others table set (gelu/tanh/square/copy):
  sigmoid(g) is computed as 0.5 + 0.5*tanh(g/2) folded into the GLU math,
  so only ~2 ACT table loads happen for the whole kernel.
"""

import numpy as np

import concourse.bacc as bacc
import concourse.tile as tile
from concourse import mybir
from concourse import bass_utils

# ---- problem constants (hardcoded per contract) ----
B, L, D, S = 8, 2048, 512, 256
DFF = 2 * D
LN_EPS = 1e-5
N_CORES = 8

# ---- tiling ----
P = 128
TC = 512                 # time chunk
NCHUNK = L // TC         # 4
KD = D // P              # 4  k-tiles over d
KS = S // P              # 2  k-tiles over s
MD = D // P              # 4  m-tiles over d outputs

F32 = mybir.dt.float32
F16 = mybir.dt.float16
AOP = mybir.AluOpType
AF = mybir.ActivationFunctionType
NP16 = np.float16


def _pack_rb(r):
    """[P, KS, TC] broadcast decay, with r=0 at the fused-scan boundary
    (s-tile 1, t=0) so the 1024-wide scan resets there; the true carry is
    injected into data1 instead."""
    rb = np.broadcast_to(r.reshape(KS, P, 1), (KS, P, TC)).transpose(1, 0, 2).copy()
    rb[:, 1, 0] = 0.0
    return np.ascontiguousarray(rb).astype(np.float32)


def _pack_kpm(w, k_tiles, m):
    """[K, M] -> [128, k_tiles, M] host pack for lhsT storage (K = kt*128+p)."""
    K = k_tiles * P
    assert w.shape == (K, m)
    return np.ascontiguousarray(w.reshape(k_tiles, P, m).transpose(1, 0, 2))


def _build(nc, with_bc=False, with_ba=False):
    f32 = F32
    f16 = F16

    xT = nc.dram_tensor("xT", [P, KD, L], f32, kind="ExternalInput")
    xT16 = nc.dram_tensor("xT16", [P, KD, L], f16, kind="ExternalInput")
    bt_re = nc.dram_tensor("bt_re", [P, KD, S], f16, kind="ExternalInput")
    bt_im = nc.dram_tensor("bt_im", [P, KD, S], f16, kind="ExternalInput")
    ct_re = nc.dram_tensor("ct_re", [P, KS, D], f16, kind="ExternalInput")
    ct_imn = nc.dram_tensor("ct_imn", [P, KS, D], f16, kind="ExternalInput")
    dt_w = nc.dram_tensor("dt_w", [P, KD, D], f16, kind="ExternalInput")
    wt = nc.dram_tensor("wt", [P, KD, DFF], f16, kind="ExternalInput")
    cosT = nc.dram_tensor("cosT", [P, KS, L], f16, kind="ExternalInput")
    sinT = nc.dram_tensor("sinT", [P, KS, L], f16, kind="ExternalInput")
    r_b = nc.dram_tensor("r_b", [P, KS, TC], f32, kind="ExternalInput")
    r_col = nc.dram_tensor("r_col", [P, KS], f32, kind="ExternalInput")
    bc_re = nc.dram_tensor("bc_re", [P, KS], f32, kind="ExternalInput")
    bc_im = nc.dram_tensor("bc_im", [P, KS], f32, kind="ExternalInput")
    gbias = nc.dram_tensor("gbias", [P, MD], f32, kind="ExternalInput")
    b_a = nc.dram_tensor("b_a", [P, MD], f32, kind="ExternalInput")
    b_gh = nc.dram_tensor("b_gh", [P, MD], f32, kind="ExternalInput")
    outT = nc.dram_tensor("outT", [P, KD, L], f32, kind="ExternalOutput")

    with tile.TileContext(nc) as tc:
        with (
            tc.tile_pool(name="wpool", bufs=1) as wpool,
            tc.tile_pool(name="io", bufs=2) as io,
            tc.tile_pool(name="work", bufs=1) as work,
            tc.tile_pool(name="carry", bufs=2) as carry_pool,
            tc.tile_pool(name="psum", bufs=1, space="PSUM") as psum,
        ):
            # ---- x16 first: the stats pipeline depends on it ----
            x16_sb = wpool.tile([P, KD, L], f16)
            for ck in range(NCHUNK):
                nc.sync.dma_start(
                    x16_sb[:, :, ck * TC : (ck + 1) * TC],
                    xT16[:, :, ck * TC : (ck + 1) * TC],
                )

            # ---- resident weights/constants ----
            w_bt_re = wpool.tile([P, KD, S], f16)
            nc.sync.dma_start(w_bt_re[:], bt_re[:])
            w_bt_im = wpool.tile([P, KD, S], f16)
            nc.sync.dma_start(w_bt_im[:], bt_im[:])
            w_ct_re = wpool.tile([P, KS, D], f16)
            nc.sync.dma_start(w_ct_re[:], ct_re[:])
            w_ct_imn = wpool.tile([P, KS, D], f16)
            nc.sync.dma_start(w_ct_imn[:], ct_imn[:])
            w_dt = wpool.tile([P, KD, D], f16)
            nc.sync.dma_start(w_dt[:], dt_w[:])
            w_wt = wpool.tile([P, KD, DFF], f16)
            nc.sync.dma_start(w_wt[:], wt[:])
            w_r = wpool.tile([P, KS, TC], f32)
            nc.sync.dma_start(w_r[:], r_b[:])
            w_rcol = wpool.tile([P, KS], f32)
            nc.sync.dma_start(w_rcol[:], r_col[:])
            w_bc_re = wpool.tile([P, KS], f32)
            nc.sync.dma_start(w_bc_re[:], bc_re[:])
            w_bc_im = wpool.tile([P, KS], f32)
            nc.sync.dma_start(w_bc_im[:], bc_im[:])
            w_gbias = wpool.tile([P, MD], f32)
            nc.sync.dma_start(w_gbias[:], gbias[:])
            w_ba = wpool.tile([P, MD], f32)
            nc.sync.dma_start(w_ba[:], b_a[:])
            w_bgh = wpool.tile([P, MD], f32)
            nc.sync.dma_start(w_bgh[:], b_gh[:])
            ones = wpool.tile([P, P], f16)
            nc.vector.memset(ones, 1.0)
            w_eps = wpool.tile([P, 1], f32)
            nc.vector.memset(w_eps, LN_EPS)

            # ---- phase 0: LN stats for all chunks ----
            mu16_all = wpool.tile([P, NCHUNK, TC], f16)
            var_all = wpool.tile([P, NCHUNK, TC], f32)
            for ck in range(NCHUNK):
                t0 = ck * TC
                x2_sb = work.tile([P, KD, TC], f16, tag="x2", bufs=2)
                for kt in range(KD):
                    nc.scalar.activation(
                        x2_sb[:, kt, :], x16_sb[:, kt, t0 : t0 + TC], AF.Square
                    )
                mu_ps = psum.tile([P, TC], f32, tag="pj", bufs=4, name=f"mu{ck}")
                msq_ps = psum.tile([P, TC], f32, tag="pj", bufs=4, name=f"msq{ck}")
                for kt in range(KD):
                    nc.tensor.matmul(
                        mu_ps[:], lhsT=ones[:], rhs=x16_sb[:, kt, t0 : t0 + TC],
                        start=(kt == 0), stop=(kt == KD - 1),
                    )
                for kt in range(KD):
                    nc.tensor.matmul(
                        msq_ps[:], lhsT=ones[:], rhs=x2_sb[:, kt, :],
                        start=(kt == 0), stop=(kt == KD - 1),
                    )
                # mu' (fp16, for xc) and var = msq/D - mu'^2
                nc.scalar.activation(
                    mu16_all[:, ck, :], mu_ps[:], AF.Copy, scale=1.0 / D
                )
                mu2 = work.tile([P, TC], f32, tag="mu2")
                nc.scalar.activation(mu2[:], mu_ps[:], AF.Square, scale=1.0 / D)
                nc.vector.scalar_tensor_tensor(
                    var_all[:, ck, :], msq_ps[:], 1.0 / D, mu2[:],
                    op0=AOP.mult, op1=AOP.subtract,
                )
            # sigma = sqrt(var + eps) over all chunks at once, then
            # rstd = 1/sigma via the fast custom-DVE reciprocal (~18 bits).
            sigma_all = work.tile([P, NCHUNK * TC], f32, tag="sigma", bufs=1)
            nc.scalar.activation(
                sigma_all[:], var_all.rearrange("p c t -> p (c t)"),
                AF.Sqrt, bias=w_eps[:],
            )
            rstd32 = work.tile([P, NCHUNK * TC], f32, tag="rstd32", bufs=1)
            nc.vector.reciprocal_approx_fast(rstd32[:], sigma_all[:])
            rstd16_all = wpool.tile([P, NCHUNK, TC], f16)
            nc.scalar.activation(
                rstd16_all.rearrange("p c t -> p (c t)"), rstd32[:], AF.Copy
            )
            # xhat for the whole sequence (feeds Bu and D matmuls directly)
            xhat_all = wpool.tile([P, KD, L], f16)
            for ck in range(NCHUNK):
                t0 = ck * TC
                mu_b = mu16_all[:, ck : ck + 1, :].broadcast_to((P, KD, TC))
                rs_b = rstd16_all[:, ck : ck + 1, :].broadcast_to((P, KD, TC))
                xc = work.tile([P, KD, TC], f16, tag="xc", bufs=2)
                nc.vector.tensor_sub(
                    xc[:], x16_sb[:, :, t0 : t0 + TC], mu_b
                )
                nc.vector.tensor_mul(
                    xhat_all[:, :, t0 : t0 + TC], xc[:], rs_b
                )

            # ---- main loop over time chunks ----
            u_prev = None
            for ck in range(NCHUNK):
                t0 = ck * TC

                x_sb = io.tile([P, KD, TC], f32, tag="x")
                nc.sync.dma_start(x_sb[:], xT[:, :, t0 : t0 + TC])
                cos_sb = io.tile([P, KS, TC], f16, tag="cos")
                nc.sync.dma_start(cos_sb[:], cosT[:, :, t0 : t0 + TC])
                sin_sb = io.tile([P, KS, TC], f16, tag="sin")
                nc.sync.dma_start(sin_sb[:], sinT[:, :, t0 : t0 + TC])

                # ---- Bu matmuls -> psum (4 banks: [re/im] x [s-tile]) ----
                ps_bu = [
                    [
                        psum.tile([P, TC], f32, tag="bu", bufs=2, name=f"bu{c}{st}")
                        for st in range(KS)
                    ]
                    for c in range(2)
                ]
                for st in range(KS):
                    for comp, w_bt in ((0, w_bt_re), (1, w_bt_im)):
                        for kt in range(KD):
                            nc.tensor.matmul(
                                ps_bu[comp][st][:],
                                lhsT=w_bt[:, kt, st * P : (st + 1) * P],
                                rhs=xhat_all[:, kt, t0 : t0 + TC],
                                start=(kt == 0),
                                stop=(kt == KD - 1),
                            )

                # ---- evac Bu (+ state bias bc = B_norm @ ln_b) to fp16 ----
                bu_re = work.tile([P, KS, TC], f16, tag="bu_re")
                bu_im = work.tile([P, KS, TC], f16, tag="bu_im")
                for st in range(KS):
                    if with_bc:
                        nc.vector.tensor_scalar_add(
                            bu_re[:, st, :], ps_bu[0][st][:],
                            w_bc_re[:, st : st + 1],
                        )
                        nc.vector.tensor_scalar_add(
                            bu_im[:, st, :], ps_bu[1][st][:],
                            w_bc_im[:, st : st + 1],
                        )
                    else:
                        nc.scalar.activation(
                            bu_re[:, st, :], ps_bu[0][st][:], AF.Copy)
                        nc.scalar.activation(
                            bu_im[:, st, :], ps_bu[1][st][:], AF.Copy)

                # ---- twiddle: c = e^{-i theta t} * Bu ----
                c_re = work.tile([P, KS, TC], f16, tag="c_re")
                c_im = work.tile([P, KS, TC], f16, tag="c_im")
                tw1 = work.tile([P, KS, TC], f16, tag="tw1")
                tw2 = work.tile([P, KS, TC], f16, tag="tw2")
                fl = lambda t: t.rearrange("p s t -> p (s t)")
                nc.vector.tensor_mul(fl(tw1), fl(cos_sb), fl(bu_re))
                nc.vector.tensor_mul(fl(tw2), fl(sin_sb), fl(bu_im))
                nc.vector.tensor_add(fl(c_re), fl(tw1), fl(tw2))
                nc.vector.tensor_mul(fl(tw1), fl(cos_sb), fl(bu_im))
                nc.vector.tensor_mul(fl(tw2), fl(sin_sb), fl(bu_re))
                nc.vector.tensor_sub(fl(c_im), fl(tw1), fl(tw2))

                # ---- scans: u_t = r*u_{t-1} + c_t, both s-tiles fused in
                # one 1024-wide scan per component (r=0 at the boundary
                # resets the state; the true s1 carry is injected into c) ----
                u = carry_pool.tile([P, 2, KS, TC], f16, tag="u")
                for comp, c_t in ((0, c_re), (1, c_im)):
                    if u_prev is not None:
                        nc.vector.scalar_tensor_tensor(
                            c_t[:, 1, 0:1],
                            u_prev[:, comp, 1, TC - 1 : TC],
                            w_rcol[:, 1:2],
                            c_t[:, 1, 0:1],
                            op0=AOP.mult, op1=AOP.add,
                        )
                        init = u_prev[:, comp, 0, TC - 1 : TC]
                    else:
                        init = 0.0
                    nc.vector.tensor_tensor_scan(
                        u[:, comp, :, :].rearrange("p s t -> p (s t)"),
                        w_r.rearrange("p s t -> p (s t)"),
                        c_t.rearrange("p s t -> p (s t)"),
                        init,
                        op0=AOP.mult,
                        op1=AOP.add,
                    )
                u_prev = u

                # ---- untwiddle: xs = e^{+i theta t} u ----
                # s-tile 0 on VectorE, s-tile 1 on GpSimd (load balance)
                xs_re = work.tile([P, KS, TC], f16, tag="xs_re")
                xs_im = work.tile([P, KS, TC], f16, tag="xs_im")
                u_re = u[:, 0, :, :].rearrange("p s t -> p (s t)")
                u_im = u[:, 1, :, :].rearrange("p s t -> p (s t)")
                nc.vector.tensor_mul(fl(tw1), fl(cos_sb), u_re)
                nc.vector.tensor_mul(fl(tw2), fl(sin_sb), u_im)
                nc.vector.tensor_sub(fl(xs_re), fl(tw1), fl(tw2))
                nc.vector.tensor_mul(fl(tw1), fl(sin_sb), u_re)
                nc.vector.tensor_mul(fl(tw2), fl(cos_sb), u_im)
                nc.vector.tensor_add(fl(xs_im), fl(tw1), fl(tw2))

                # ---- y = C_re@xs_re + (-C_im)@xs_im + (D.w)@xhat -> gelu ----
                h_sb = work.tile([P, MD, TC], f16, tag="h")
                for mt in range(MD):
                    ps_y = psum.tile([P, TC], f32, tag="y", bufs=2, name=f"y{mt}")
                    for kt in range(KD):
                        nc.tensor.matmul(
                            ps_y[:],
                            lhsT=w_dt[:, kt, mt * P : (mt + 1) * P],
                            rhs=xhat_all[:, kt, t0 : t0 + TC],
                            start=(kt == 0), stop=False,
                        )
                    for st in range(KS):
                        nc.tensor.matmul(
                            ps_y[:],
                            lhsT=w_ct_re[:, st, mt * P : (mt + 1) * P],
                            rhs=xs_re[:, st, :],
                            start=False, stop=False,
                        )
                    for st in range(KS):
                        nc.tensor.matmul(
                            ps_y[:],
                            lhsT=w_ct_imn[:, st, mt * P : (mt + 1) * P],
                            rhs=xs_im[:, st, :],
                            start=False, stop=(st == KS - 1),
                        )
                    nc.scalar.activation(
                        h_sb[:, mt, :], ps_y[:], AF.Gelu,
                        bias=w_gbias[:, mt : mt + 1],
                    )

                # ---- proj = W.h ; GLU via tanh ; residual ----
                # sigmoid(g+bg) = 0.5 + 0.5*tanh((g+bg)/2)
                # out = (a+ba)*sig + x = 0.5*[(a+ba) + (a+ba)*t] + x
                out_sb = io.tile([P, KD, TC], f32, tag="out")
                for mt in range(MD):
                    th = work.tile([P, TC], f16, tag="th", bufs=2)
                    w16 = work.tile([P, TC], f16, tag="w16", bufs=2)
                    q = work.tile([P, TC], f32, tag="q", bufs=2)
                    ps_pa = psum.tile([P, TC], f32, tag="pj", bufs=4, name=f"pa{mt}")
                    ps_pg = psum.tile([P, TC], f32, tag="pj", bufs=4, name=f"pg{mt}")
                    for kt in range(KD):
                        nc.tensor.matmul(
                            ps_pa[:],
                            lhsT=w_wt[:, kt, mt * P : (mt + 1) * P],
                            rhs=h_sb[:, kt, :],
                            start=(kt == 0), stop=(kt == KD - 1),
                        )
                    for kt in range(KD):
                        nc.tensor.matmul(
                            ps_pg[:],
                            lhsT=w_wt[:, kt, D + mt * P : D + (mt + 1) * P],
                            rhs=h_sb[:, kt, :],
                            start=(kt == 0), stop=(kt == KD - 1),
                        )
                    nc.scalar.activation(
                        th[:], ps_pg[:], AF.Tanh,
                        bias=w_bgh[:, mt : mt + 1], scale=0.5,
                    )
                    nc.vector.tensor_scalar_add(w16[:], th[:], 1.0)
                    if with_ba:
                        nc.vector.scalar_tensor_tensor(
                            q[:], ps_pa[:], w_ba[:, mt : mt + 1],
                            w16[:], op0=AOP.add, op1=AOP.mult,
                        )
                    else:
                        a16 = work.tile([P, TC], f16, tag="a16", bufs=2)
                        nc.scalar.activation(a16[:], ps_pa[:], AF.Copy)
                        nc.vector.tensor_mul(q[:], a16[:], w16[:])
                    nc.vector.tensor_add(
                        out_sb[:, mt, :], q[:], x_sb[:, mt, :]
                    )

                nc.sync.dma_start(outT[:, :, t0 : t0 + TC], out_sb[:])

    nc.compile()
    return nc


_NC_CACHE = {}


def _get_module(with_bc=False, with_ba=False):
    key = (with_bc, with_ba)
    if key not in _NC_CACHE:
        nc = bacc.Bacc("TRN2", target_bir_lowering=False, debug=False)
        _NC_CACHE[key] = _build(nc, with_bc=with_bc, with_ba=with_ba)
    return _NC_CACHE[key]


def _host_prepack(inputs):
    ln_w = np.asarray(inputs["ln_w"], np.float64)
    ln_b = np.asarray(inputs["ln_b"], np.float64)
    nu_log = np.asarray(inputs["nu_log"], np.float64)
    theta_log = np.asarray(inputs["theta_log"], np.float64)
    gamma_log = np.asarray(inputs["gamma_log"], np.float64)
    B_re = np.asarray(inputs["B_re"], np.float64)
    B_im = np.asarray(inputs["B_im"], np.float64)
    C_re = np.asarray(inputs["C_re"], np.float64)
    C_im = np.asarray(inputs["C_im"], np.float64)
    D_m = np.asarray(inputs["D"], np.float64)
    W_out = np.asarray(inputs["W_out"], np.float64)
    b_out = np.asarray(inputs["b_out"], np.float64)

    r = np.exp(-np.exp(nu_log))
    theta = np.exp(theta_log)
    g = np.exp(gamma_log)
    ang = theta[:, None] * np.arange(L, dtype=np.float64)[None, :]
    cos_t = np.cos(ang)
    sin_t = np.sin(ang)

    Bn_re = B_re * g[:, None]
    Bn_im = B_im * g[:, None]
    BnT_re = (Bn_re * ln_w[None, :]).T
    BnT_im = (Bn_im * ln_w[None, :]).T
    bc_re_v = Bn_re @ ln_b
    bc_im_v = Bn_im @ ln_b
    CT_re = C_re.T
    CT_imn = (-C_im).T
    DT = (D_m * ln_w[None, :]).T
    gbias_v = D_m @ ln_b
    WT = W_out.T.copy()
    WT[:, :D] *= 0.5
    b_a_v = 0.5 * b_out[:D]
    b_gh_v = 0.5 * b_out[D:]

    def cols(v, ntiles):
        return np.ascontiguousarray(np.asarray(v, np.float32).reshape(ntiles, P).T)

    return {
        "bt_re": _pack_kpm(BnT_re, KD, S).astype(NP16),
        "bt_im": _pack_kpm(BnT_im, KD, S).astype(NP16),
        "ct_re": _pack_kpm(CT_re, KS, D).astype(NP16),
        "ct_imn": _pack_kpm(CT_imn, KS, D).astype(NP16),
        "dt_w": _pack_kpm(DT, KD, D).astype(NP16),
        "wt": _pack_kpm(WT, KD, DFF).astype(NP16),
        "cosT": np.ascontiguousarray(
            cos_t.reshape(KS, P, L).transpose(1, 0, 2)
        ).astype(NP16),
        "sinT": np.ascontiguousarray(
            sin_t.reshape(KS, P, L).transpose(1, 0, 2)
        ).astype(NP16),
        "r_b": _pack_rb(r),
        "r_col": np.ascontiguousarray(r.reshape(KS, P).T).astype(np.float32),
        "bc_re": cols(bc_re_v, KS),
        "bc_im": cols(bc_im_v, KS),
        "gbias": cols(gbias_v, MD),
        "b_a": cols(b_a_v, MD),
        "b_gh": cols(b_gh_v, MD),
    }


def _make_in_maps(inputs):
    x = np.asarray(inputs["x"], np.float32)
    weights = _host_prepack(inputs)
    in_maps = []
    for b in range(B):
        xb = np.ascontiguousarray(x[b].T.reshape(KD, P, L).transpose(1, 0, 2))
        m = dict(weights)
        m["xT"] = xb
        m["xT16"] = xb.astype(NP16)
        in_maps.append(m)
    return in_maps


def kernel(**inputs):
    in_maps = _make_in_maps(inputs)
    with_bc = bool(np.any(np.asarray(inputs["ln_b"]) != 0))
    with_ba = bool(np.any(np.asarray(inputs["b_out"]) != 0))
    nc = _get_module(with_bc, with_ba)
    res = bass_utils.run_bass_kernel_spmd(nc, in_maps, core_ids=list(range(N_CORES)))
    out = np.empty((B, L, D), np.float32)
    for b in range(B):
        ob = res.results[b]["outT"]
        out[b] = ob.transpose(1, 0, 2).reshape(D, L).T
    return out
